# revision 1
# baseline (speedup 1.0000x reference)
"""Trainium2 Bass kernel for nn_Net_90331752170289 (Mamba block + FFT/CNN + fusion head).

Strategy: pure data parallelism over batch (8 batches per core on 8 cores).
Per-core layout: partitions carry (batch, channel) blocks padded to the
32-partition grid, free dim = time (2048 per batch).  The Mamba selective
scan runs as hardware tensor_tensor_scan instructions over (d,n)-partition
tiles in bf16; the FFT is a DFT matmul in fp16 against host-built cos/sin
matrices (half spectrum + mirror via reversed-identity PE transposes); all
small linear layers are block-diagonal float32r matmuls with LN/conv/affine
folds precomputed on the host.  Activation functions are composed from the
two ACT tables (sigmoid/erf and ln/exp) to avoid table thrashing.

Block layouts (per core, 8 local batches b, groups g=b//4, bi=b%4):
  X24 tensors (xi, siluz, delta, dx, y): [128, 2048] per g, row = 32*bi + ch
  BC: [128, 2048] per g, row = 32*bi + r (r<16 B, else C)
  X12 tensors (hhat, h_aff, s_t, xm_hat, xcnn): [128, 2048], row = 16*b + m
  scan tiles: [128, 2048] per (b, dn-tile), row = 16*dl + n, d = 8*tile + dl
"""
import numpy as np

B, L, DM = 64, 2048, 12
DI, DS, DC = 24, 16, 4
NCORES = 8
BL = B // NCORES          # 8 local batches per core
N = L                     # free dim per batch
NCH = 1024                # psum chunk (2 banks)
NCY = 512                 # scan-output psum chunk
NF = 1152                 # padded rfft bins (valid 0..1024)
NMT = NF // 128           # 9 DFT M-tiles
NKT = L // 128            # 16 DFT K-tiles
SQ2I = 0.7071067811865476

_CACHE = {}


# ---------------------------------------------------------------- device code
def _build_module():
    import concourse.bacc as bacc
    import concourse.bass as bass
    import concourse.tile as tile
    from concourse import mybir
    from contextlib import ExitStack

    F32 = mybir.dt.float32
    F32R = mybir.dt.float32r
    F16 = mybir.dt.float16
    BF16 = mybir.dt.bfloat16
    AF = mybir.ActivationFunctionType
    OP = mybir.AluOpType
    AX = mybir.AxisListType

    nc = bacc.Bacc("TRN2", target_bir_lowering=False, debug=False)

    def din(name, shape, dt=F32R):
        return nc.dram_tensor(name, shape, dt, kind="ExternalInput")

    # per-core data
    xs_d = din("xs", [4, 96, N], BF16)                  # in_proj rhs, per b-pair
    xt_d = din("xt", [128, NKT * 96], F16)        # DFT rhs, kt-major cols
    wdft_d = din("wdft", [NMT, 2, 128, NKT * 128], F16)
    # folded weights (identical on all cores)
    w_xc_d = din("w_xc", [96, 64], BF16)
    w_z_d = din("w_z", [96, 64], BF16)
    w_delta_d = din("w_delta", [128, 128], BF16)
    w_bc_d = din("w_bc", [128, 128], BF16)
    w_op_d = din("w_op", [128, 64])
    w_ones12_d = din("w_ones12", [128, 8])
    w_bc8_d = din("w_bc8", [8, 128])
    w_ffn1_d = din("w_ffn1", [4, 128, 128])
    w_ffn2_d = din("w_ffn2", [4, 128, 32])
    w_pc_d = din("w_pc", [128, 128])
    w_lin1a_d = din("w_lin1a", [128, 128])
    w_lin1b_d = din("w_lin1b", [128, 128])
    w_lin2_d = din("w_lin2", [2, 128, 128])
    w_lin3_d = din("w_lin3", [2, 128, 4])
    w_cnn_d = din("w_cnn", [3, 96, 128], F16)
    w_mask_d = din("w_mask", [3, 128, 32], BF16)
    sc_negA_d = din("sc_negA", [3, 128, 1], F32)
    ident_d = din("ident", [128, 128], F16)
    identj_d = din("identj", [128, 128], F16)
    vec_d = din("vecs", [128, 11], F32)           # packed per-partition vectors
    w_dp_d = din("w_dp", [4, 128, 32], BF16)
    b_out_d = din("b_out", [8, 1], F32)
    b_eps_d = din("b_eps", [8, 1], F32)
    (V_BCONV, V_BDT, V_SDP, V_G1, V_B1, V_BFFN1, V_BFFN2, V_BHEAD1,
     V_BLIN2, V_BCNN, V_BFFN1S) = range(11)

    out_d = nc.dram_tensor("out", [8, 1], F32, kind="ExternalOutput")

    with tile.TileContext(nc) as tc, ExitStack() as ctx:
        sg = ctx.enter_context(tc.tile_pool(name="singles", bufs=1))
        ws = ctx.enter_context(tc.tile_pool(name="work", bufs=2))
        big = ctx.enter_context(tc.tile_pool(name="big", bufs=1))
        ring = ctx.enter_context(tc.tile_pool(name="ring", bufs=3))
        pmm = ctx.enter_context(tc.tile_pool(name="pmm", bufs=2, space="PSUM"))
        py = ctx.enter_context(tc.tile_pool(name="py", bufs=2, space="PSUM"))
        pdft = ctx.enter_context(tc.tile_pool(name="pdft", bufs=2, space="PSUM"))

        def load(dram_ap, shape, dt, tag, pool=sg):
            t = pool.tile(shape, dt, tag=tag, name=tag)
            nc.sync.dma_start(out=t, in_=dram_ap)
            return t

        from concourse.tile_rust import add_dep_helper
        _last_act = [None]

        def act(out, in_, func, **kw):
            inst = nc.scalar.activation(out, in_, func, **kw)
            if not _CACHE.get("act_chain", False):
                return inst
            if _last_act[0] is not None:
                add_dep_helper(inst.ins, _last_act[0],
                               reason="act-table ordering chain")
            _last_act[0] = inst.ins
            return inst

        def mm512(p, lhsT, rhs, c0, c1, start=True, stop=True):
            # matmul into a [*, c1-c0] psum tile in 512-wide pieces
            for o in range(0, c1 - c0, 512):
                w = min(512, c1 - c0 - o)
                nc.tensor.matmul(p[:, o:o + w], lhsT, rhs[:, c0 + o:c0 + o + w],
                                 start=start, stop=stop)

        # ---- load weights/constants into SBUF
        W_xc = load(w_xc_d[:, :], [96, 64], BF16, "w_xc")
        W_z = load(w_z_d[:, :], [96, 64], BF16, "w_z")
        W_delta = load(w_delta_d[:, :], [128, 128], BF16, "w_delta")
        W_bc = load(w_bc_d[:, :], [128, 128], BF16, "w_bc")
        W_op = load(w_op_d[:, :], [128, 64], F32R, "w_op")
        W_ones12 = load(w_ones12_d[:, :], [128, 8], F32R, "w_ones12")
        W_bc8 = load(w_bc8_d[:, :], [8, 128], F32R, "w_bc8")
        W_ffn1 = [load(w_ffn1_d[q], [128, 128], F32R, f"w_ffn1_{q}")
                  for q in range(4)]
        W_ffn2 = [load(w_ffn2_d[q], [128, 32], F32R, f"w_ffn2_{q}")
                  for q in range(4)]
        W_pc = load(w_pc_d[:, :], [128, 128], F32R, "w_pc")
        W_lin1a = load(w_lin1a_d[:, :], [128, 128], F32R, "w_lin1a")
        W_lin1b = load(w_lin1b_d[:, :], [128, 128], F32R, "w_lin1b")
        W_lin2 = [load(w_lin2_d[g], [128, 128], F32R, f"w_lin2_{g}")
                  for g in range(2)]
        W_lin3 = [load(w_lin3_d[g], [128, 4], F32R, f"w_lin3_{g}")
                  for g in range(2)]
        W_cnn = [load(w_cnn_d[k], [96, 128], F16, f"w_cnn_{k}")
                 for k in range(3)]
        W_mask = [load(w_mask_d[t], [128, 32], BF16, f"w_mask_{t}")
                  for t in range(3)]
        ScA = [load(sc_negA_d[t], [128, 1], F32, f"scA_{t}") for t in range(3)]
        Ident = load(ident_d[:, :], [128, 128], F16, "ident")
        IdentJ = load(identj_d[:, :], [128, 128], F16, "identj")
        Vec_t = load(vec_d[:, :], [128, 11], F32, "vec_t")
        Vec = [Vec_t[:, i:i + 1] for i in range(11)]
        W_dp = [load(w_dp_d[bi], [128, 32], BF16, f"w_dp{bi}")
                for bi in range(4)]
        Bout = load(b_out_d[:, :], [8, 1], F32, "b_out")
        Beps = load(b_eps_d[:, :], [8, 1], F32, "b_eps")

        # ---- persistent activations
        xi = [big.tile([128, N], BF16, tag=f"xi{g}", name=f"xi{g}")
              for g in range(2)]
        siluz = [big.tile([128, N], BF16, tag=f"siluz{g}", name=f"siluz{g}")
                 for g in range(2)]
        delta = [big.tile([128, N], BF16, tag=f"delta{g}", name=f"delta{g}")
                 for g in range(2)]
        dx = [big.tile([128, N], BF16, tag=f"dx{g}", name=f"dx{g}")
              for g in range(2)]
        BC = [big.tile([128, N], BF16, tag=f"bc{g}", name=f"bc{g}")
              for g in range(2)]
        y = [ring.tile([128, N], F32R, tag="r8", name=f"y{g}")
             for g in range(2)]
        xcnn = big.tile([128, N], F32R, tag="xcnn", name="xcnn")
        xfT = big.tile([96, N + 2], F16, tag="xfT", name="xfT")

        CH = [(c * NCH, (c + 1) * NCH) for c in range(N // NCH)]

        # ================= phase A: fused in_proj + causal conv, silu =======
        for g in range(2):
            xsp = [ws.tile([96, N], BF16, tag="pairA", name="xsp")
                   for j in range(2)]
            for j in range(2):
                nc.sync.dma_start(out=xsp[j], in_=xs_d[2 * g + j])
            for c0, c1 in CH:
                p_xc = [pmm.tile([64, NCH], F32, tag="pmm", name="pmm")
                        for j in range(2)]
                p_z = [pmm.tile([64, NCH], F32, tag="pmm", name="pmm")
                       for j in range(2)]
                for j in range(2):
                    mm512(p_xc[j], W_xc, xsp[j], c0, c1)
                    mm512(p_z[j], W_z, xsp[j], c0, c1)
                for j in range(2):
                    jj = slice(64 * j, 64 * j + 64)
                    act(xi[g][jj, c0:c1], p_xc[j], AF.Silu,
                        bias=Vec[V_BCONV][jj, :])
                    act(siluz[g][jj, c0:c1], p_z[j], AF.Silu)

        # ================= phase B: x_proj (delta folded), dx ==============
        for g in range(2):
            for c0, c1 in CH:
                p_d = pmm.tile([128, NCH], F32, tag="pmm", name="pmm")
                mm512(p_d, W_delta, xi[g], c0, c1)
                edt = ws.tile([128, NCH], F32, tag="sgm", name="edt")
                act(edt, p_d, AF.Exp, bias=Vec[V_BDT])
                act(delta[g][:, c0:c1], edt, AF.Ln, bias=1.0)
                p_bc = pmm.tile([128, NCH], F32, tag="pmm", name="pmm")
                mm512(p_bc, W_bc, xi[g], c0, c1)
                act(BC[g][:, c0:c1], p_bc, AF.Copy)
            nc.vector.tensor_mul(dx[g], delta[g], xi[g])

        # ================= phase C: selective scan ==========================
        for b in range(BL):
            g, bi = b // 4, b % 4
            # B/C broadcast: tile the per-b [16,N] rows 8x across partitions
            Bbc = ws.tile([128, N], BF16, tag="pairA", name="Bbc")
            Cbc = ws.tile([128, N], BF16, tag="pairA", name="Cbc")
            nc.gpsimd.dma_start(out=Bbc[0:16, :],
                                in_=BC[g][32 * bi:32 * bi + 16, :])
            nc.gpsimd.dma_start(out=Cbc[0:16, :],
                                in_=BC[g][32 * bi + 16:32 * bi + 32, :])
            for r in (16, 32, 64):
                nc.gpsimd.dma_start(out=Bbc[r:2 * r, :], in_=Bbc[0:r, :])
                nc.gpsimd.dma_start(out=Cbc[r:2 * r, :], in_=Cbc[0:r, :])
            hcs = []
            for t in range(3):
                r0 = 32 * bi + 8 * t
                dl_sl = delta[g][r0:r0 + 8, :]
                dbc = ws.tile([128, N], BF16, tag="dbc", name="dbc")
                nc.sync.dma_start(
                    out=dbc,
                    in_=bass.AP(tensor=dl_sl.tensor, offset=dl_sl.offset,
                                ap=[dl_sl.ap[0], [0, 16], dl_sl.ap[1]]))
                a_t = ws.tile([128, N], BF16, tag="a_t", name="a_t")
                act(a_t, dbc, AF.Exp, scale=ScA[t])
                dx_sl = dx[g][r0:r0 + 8, :]
                dxbc = ws.tile([128, N], BF16, tag="dxbc", name="dxbc")
                nc.sync.dma_start(
                    out=dxbc,
                    in_=bass.AP(tensor=dx_sl.tensor, offset=dx_sl.offset,
                                ap=[dx_sl.ap[0], [0, 16], dx_sl.ap[1]]))
                dBx = ws.tile([128, N], BF16, tag="dbc", name="dBx")
                nc.vector.tensor_mul(dBx, dxbc, Bbc)
                h_t = ws.tile([128, N], BF16, tag="dxbc", name="h_t")
                nc.vector.tensor_tensor_scan(h_t, a_t, dBx, 0.0,
                                             OP.mult, OP.add)
                hc = ws.tile([128, N], BF16, tag=f"hc{t}", name="hc", bufs=1)
                for c0 in range(0, N, NCY):
                    nc.vector.tensor_mul(hc[:, c0:c0 + NCY],
                                         h_t[:, c0:c0 + NCY],
                                         Cbc[:, c0:c0 + NCY])
                hcs.append(hc)
            # y = (ys + xi*Dp) * silu(z)
            rr = slice(32 * bi, 32 * bi + 32)
            for c0 in range(0, N, NCY):
                c1 = c0 + NCY
                p_yt = py.tile([32, NCY], F32, tag="pyt", name="pyt")
                for t in range(3):
                    nc.tensor.matmul(p_yt, W_mask[t], hcs[t][:, c0:c1],
                                     start=(t == 0), stop=False)
                nc.tensor.matmul(p_yt, W_dp[bi], xi[g][:, c0:c1],
                                 start=False, stop=True)
                nc.vector.tensor_mul(y[g][rr, c0:c1], p_yt,
                                     siluz[g][rr, c0:c1])

        # ================= phase D: out_proj -> LN1 -> FFN -> LN2 ==========
        hhat = ring.tile([128, N], F32R, tag="r8", name="hhat")
        h_aff = ring.tile([128, N], F32R, tag="r8", name="h_aff")
        for c0, c1 in CH:
            p_m = [pmm.tile([64, NCH], F32, tag="pmm", name="pmm")
                   for g in range(2)]
            for g in range(2):
                mm512(p_m[g], W_op, y[g], c0, c1)
            cent = ws.tile([128, NCH], F32R, tag="cent", name="cent")
            sq = ws.tile([128, NCH], F32R, tag="sq", name="sq")
            for g in range(2):
                gg = slice(64 * g, 64 * g + 64)
                act(cent[gg, :], p_m[g], AF.Copy)
                act(sq[gg, :], p_m[g], AF.Square)
            p_v = pmm.tile([8, NCH], F32, tag="pmm", name="pmm")
            mm512(p_v, W_ones12, sq, 0, NCH)
            sd = ws.tile([8, NCH], F32, tag="sd", name="sd")
            act(sd, p_v, AF.Ln, bias=Beps)
            inv = ws.tile([8, NCH], F32R, tag="sd", name="inv")
            act(inv, sd, AF.Exp, scale=-0.5)
            p_b = pmm.tile([128, NCH], F32, tag="pmm", name="pmm")
            mm512(p_b, W_bc8, inv, 0, NCH)
            nc.vector.tensor_mul(hhat[:, c0:c1], cent, p_b)
            nc.vector.tensor_scalar(h_aff[:, c0:c1], hhat[:, c0:c1],
                                    Vec[V_G1], Vec[V_B1], OP.mult, OP.add)
        # FFN (chunk-wise; gelu = 0.5*u*(1+erf(u/sqrt2)), 0.5 folded in W_ffn2)
        s_t = ring.tile([128, N], F32R, tag="r8", name="s_t")
        for q in range(4):
            for c0, c1 in CH:
                p_f = pmm.tile([128, NCH], F32, tag="pmm", name="pmm")
                mm512(p_f, W_ffn1[q], hhat, c0, c1)
                erf_t = ws.tile([128, NCH], F32, tag="sgm", name="erf_t")
                act(erf_t, p_f, AF.Erf, scale=SQ2I, bias=Vec[V_BFFN1S])
                ue = ws.tile([128, NCH], F32, tag="sgm", name="ue")
                nc.vector.scalar_tensor_tensor(
                    ue, p_f, Vec[V_BFFN1], erf_t, OP.add, OP.mult)
                ff_c = ws.tile([128, NCH], F32R, tag="ffch", name="ff_c")
                nc.vector.scalar_tensor_tensor(
                    ff_c, p_f, Vec[V_BFFN1], ue, OP.add, OP.add)
                p_2 = pmm.tile([32, NCH], F32, tag="pmm", name="pmm")
                mm512(p_2, W_ffn2[q], ff_c, 0, NCH)
                rq = slice(32 * q, 32 * q + 32)
                nc.vector.scalar_tensor_tensor(
                    s_t[rq, c0:c1], p_2, Vec[V_BFFN2][rq, :],
                    h_aff[rq, c0:c1], OP.add, OP.add)
        # LN2
        xm_hat = ring.tile([128, N], F32R, tag="r8", name="xm_hat")
        for c0, c1 in CH:
            p_c = pmm.tile([128, NCH], F32, tag="pmm", name="pmm")
            mm512(p_c, W_pc, s_t, c0, c1)
            c2 = ws.tile([128, NCH], F32R, tag="cent", name="c2")
            act(c2, p_c, AF.Copy)
            sq2 = ws.tile([128, NCH], F32R, tag="sq", name="sq2")
            act(sq2, p_c, AF.Square)
            p_v2 = pmm.tile([8, NCH], F32, tag="pmm", name="pmm")
            mm512(p_v2, W_ones12, sq2, 0, NCH)
            sd2 = ws.tile([8, NCH], F32, tag="sd", name="sd2")
            act(sd2, p_v2, AF.Ln, bias=Beps)
            inv2 = ws.tile([8, NCH], F32R, tag="sd", name="inv2")
            act(inv2, sd2, AF.Exp, scale=-0.5)
            p_b2 = pmm.tile([128, NCH], F32, tag="pmm", name="pmm")
            mm512(p_b2, W_bc8, inv2, 0, NCH)
            nc.vector.tensor_mul(xm_hat[:, c0:c1], c2, p_b2)

        # ================= phase E: DFT |FFT| + CNN =========================
        xt_sb = sg.tile([128, NKT * 96], F16, tag="xt", name="xt")
        nc.sync.dma_start(out=xt_sb, in_=xt_d[:, :])
        xf = [sg.tile([128, 96], F16, tag=f"xf{m}", name=f"xf{m}")
              for m in range(NMT)]
        for mt in range(NMT):
            mags = []
            for cs in range(2):
                p_acc = pdft.tile([128, 96], F32, tag="pdft", name="pdft")
                wsl = ws.tile([128, NKT * 128], F16, tag="wsl", name="wsl")
                for hh in range(2):
                    nc.sync.dma_start(
                        out=wsl[:, 1024 * hh:1024 * hh + 1024],
                        in_=wdft_d[mt, cs, :, 1024 * hh:1024 * hh + 1024])
                for kt in range(NKT):
                    nc.tensor.matmul(p_acc, wsl[:, 128 * kt:128 * kt + 128],
                                     xt_sb[:, 96 * kt:96 * kt + 96],
                                     start=(kt == 0), stop=(kt == NKT - 1))
                m_sq = ws.tile([128, 96], F32, tag=f"m_sq{cs}", name="m_sq")
                act(m_sq, p_acc, AF.Square)
                mags.append(m_sq)
            nc.vector.scalar_tensor_tensor(mags[0], mags[0], 1e-20,
                                           mags[1], OP.add, OP.add)
            lnm = ws.tile([128, 96], F32, tag="m_sq1", name="lnm")
            act(lnm, mags[0], AF.Ln)
            act(xf[mt], lnm, AF.Exp, scale=0.5)
        # transpose + mirror into xfT [96, 2+N]: col 1+t = xf[t], cols 0/2049 0
        nc.vector.memset(xfT[:, 0:1], 0.0)
        for mt in range(NMT):
            p_t = pdft.tile([96, 128], F16, tag="pdft", name="pdft")
            nc.tensor.transpose(p_t, xf[mt], Ident)
            act(xfT[:, 1 + 128 * mt:1 + 128 * mt + 128], p_t, AF.Copy)
        for mt in range(8):        # mirrored half: t = 2048 - f, f=128*mt+j
            p_r = pdft.tile([96, 128], F16, tag="pdft", name="pdft")
            nc.tensor.transpose(p_r, xf[mt], IdentJ)
            act(xfT[:, 1922 - 128 * mt:1922 - 128 * mt + 128], p_r, AF.Copy)
        nc.vector.memset(xfT[:, N + 1:N + 2], 0.0)
        # CNN: 3 shifted block-diag matmuls
        for c0, c1 in CH:
            p_cn = pmm.tile([128, NCH], F32, tag="pmm", name="pmm")
            for k in range(3):
                mm512(p_cn, W_cnn[k], xfT, c0 + k, c1 + k,
                      start=(k == 0), stop=(k == 2))
            act(xcnn[:, c0:c1], p_cn, AF.Identity, bias=Vec[V_BCNN])

        # ================= phase F: fusion head =============================
        racc = [sg.tile([4, 1], F32, tag=f"racc{g}", name=f"racc{g}")
                for g in range(2)]
        for g in range(2):
            nc.vector.memset(racc[g], 0.0)
        for c0, c1 in CH:
            p_1 = pmm.tile([128, NCH], F32, tag="pmm", name="pmm")
            mm512(p_1, W_lin1a, xm_hat, c0, c1, start=True, stop=False)
            mm512(p_1, W_lin1b, xcnn, c0, c1, start=False, stop=True)
            mneg = ws.tile([128, NCH], F32, tag="mneg", name="mneg")
            nc.vector.tensor_scalar(mneg, p_1, Vec[V_BHEAD1], 0.0,
                                    OP.add, OP.min)
            e_t = ws.tile([128, NCH], F32, tag="e_t", name="e_t")
            act(e_t, mneg, AF.Exp)
            r_t = ws.tile([128, NCH], F32, tag="mneg", name="r_t")
            act(r_t, p_1, AF.Relu, bias=Vec[V_BHEAD1])
            v_t = ws.tile([128, NCH], F32R, tag="e_t", name="v_t")
            nc.vector.tensor_add(v_t, r_t, e_t)
            for g in range(2):
                p_o2 = pmm.tile([128, NCH], F32, tag="pmm", name="pmm")
                mm512(p_o2, W_lin2[g], v_t, 0, NCH)
                o2c = ws.tile([128, NCH], F32R, tag="mneg", name="o2c")
                act(o2c, p_o2, AF.Identity, bias=Vec[V_BLIN2])
                p_o3 = pmm.tile([4, NCH], F32, tag="pmm", name="pmm")
                mm512(p_o3, W_lin3[g], o2c, 0, NCH)
                o3c = ws.tile([4, NCH], F32, tag="sd", name="o3c")
                act(o3c, p_o3, AF.Copy)
                rc = ws.tile([4, 1], F32, tag="rc", name="rc")
                nc.vector.tensor_reduce(rc, o3c, AX.X, OP.add)
                nc.vector.tensor_add(racc[g], racc[g], rc)
        for g in range(2):
            res = sg.tile([4, 1], F32, tag=f"res{g}", name=f"res{g}")
            act(res, racc[g], AF.Sigmoid, bias=Bout[0:4, :], scale=1.0 / N)
            nc.sync.dma_start(out=out_d[4 * g:4 * g + 4, :], in_=res)

    # Prefer the combined ln+exp ACT table: hide Exp/Ln from all other
    # tables so the table-load pass lands on natural_log_exp_and_others
    # (availability-only metadata; claiming less than reality is safe).
    import concourse.bacc as bacc_mod
    from concourse import mybir as _mb
    _orig_gat = bacc_mod.get_activation_tables

    def _gat(arch):
        t = {k: set(v) for k, v in _orig_gat(arch).items()}
        for name, s in t.items():
            if name != "natural_log_exp_and_others":
                s.discard(_mb.ActivationFunctionType.Exp)
                s.discard(_mb.ActivationFunctionType.Ln)
        return t

    bacc_mod.get_activation_tables = _gat
    try:
        nc.compile()
    finally:
        bacc_mod.get_activation_tables = _orig_gat
    return nc


# ---------------------------------------------------------------- host side
def _host_prep(inputs):
    f32, f16 = np.float32, np.float16
    x = inputs["x"].astype(f32)
    in_proj_w = inputs["in_proj_w"].astype(f32)
    conv_w = inputs["conv_w"].astype(f32)
    conv_b = inputs["conv_b"].astype(f32)
    x_proj_w = inputs["x_proj_w"].astype(f32)
    dt_w = inputs["dt_w"].astype(f32)
    dt_b = inputs["dt_b"].astype(f32)
    A_log = inputs["A_log"].astype(f32)
    Dp = inputs["Dp"].astype(f32)
    out_proj_w = inputs["out_proj_w"].astype(f32)
    ln1_g, ln1_b = inputs["ln1_g"].astype(f32), inputs["ln1_b"].astype(f32)
    ffn_w1, ffn_b1 = inputs["ffn_w1"].astype(f32), inputs["ffn_b1"].astype(f32)
    ffn_w2, ffn_b2 = inputs["ffn_w2"].astype(f32), inputs["ffn_b2"].astype(f32)
    ffn_ln_g = inputs["ffn_ln_g"].astype(f32)
    ffn_ln_b = inputs["ffn_ln_b"].astype(f32)
    cnn_w, cnn_b = inputs["cnn_w"].astype(f32), inputs["cnn_b"].astype(f32)
    lin1_w, lin1_b = inputs["lin1_w"].astype(f32), inputs["lin1_b"].astype(f32)
    lin2_w, lin2_b = inputs["lin2_w"].astype(f32), inputs["lin2_b"].astype(f32)
    lin3_w, lin3_b = inputs["lin3_w"].astype(f32), inputs["lin3_b"].astype(f32)

    sh = {}
    # fused in_proj + conv:  Wxc[k*12+m, d] = conv_w[d,0,k]*in_proj_w[d,m]
    Wxc = np.einsum('dk,dm->kmd', conv_w[:, 0, :], in_proj_w[:DI]).reshape(48, DI)
    sh["w_xc"] = np.zeros((96, 64), f32)
    sh["w_z"] = np.zeros((96, 64), f32)
    for b2 in range(2):
        sh["w_xc"][48 * b2:48 * b2 + 48, 32 * b2:32 * b2 + 24] = Wxc
        for m in range(DM):
            sh["w_z"][48 * b2 + 36 + m, 32 * b2:32 * b2 + 24] = in_proj_w[DI:, m]
    # x_proj (delta rank-1 folded)
    Wdelta = np.einsum('d,j->jd', dt_w[:, 0], x_proj_w[0])     # [24,24]
    WBC = x_proj_w[1:].T                                       # [24,32]
    sh["w_delta"] = np.zeros((128, 128), f32)
    sh["w_bc"] = np.zeros((128, 128), f32)
    for bi in range(4):
        r = slice(32 * bi, 32 * bi + 24)
        sh["w_delta"][r, 32 * bi:32 * bi + 24] = Wdelta
        sh["w_bc"][r, 32 * bi:32 * bi + 32] = WBC
    # out_proj with centering fold
    Pc = np.eye(DM, dtype=f32) - f32(1.0 / DM)
    WopT = (Pc @ out_proj_w).T                                 # [24,12]
    sh["w_op"] = np.zeros((128, 64), f32)
    for bi in range(4):
        sh["w_op"][32 * bi:32 * bi + 24, 16 * bi:16 * bi + 12] = WopT
    sh["w_ones12"] = np.zeros((128, 8), f32)
    sh["w_bc8"] = np.zeros((8, 128), f32)
    for b in range(8):
        sh["w_ones12"][16 * b:16 * b + 12, b] = f32(1.0 / DM)
        sh["w_bc8"][b, 16 * b:16 * b + 16] = 1.0
    # ffn (0.5 of exact-gelu folded into w_ffn2)
    W1p = (ffn_w1 * ln1_g[None, :]).T                          # [12,48]
    b1p = ffn_b1 + ffn_w1 @ ln1_b
    sh["w_ffn1"] = np.zeros((4, 128, 128), f32)
    sh["w_ffn2"] = np.zeros((4, 128, 32), f32)
    for q in range(4):
        for b2 in range(2):
            b = 2 * q + b2
            sh["w_ffn1"][q, 16 * b:16 * b + 12, 64 * b2:64 * b2 + 48] = W1p
            sh["w_ffn2"][q, 64 * b2:64 * b2 + 48,
                         16 * b2:16 * b2 + 12] = 0.5 * ffn_w2.T
    sh["w_pc"] = np.zeros((128, 128), f32)
    W1aT = (lin1_w[:, :DM] * ffn_ln_g[None, :]).T              # [12,12]
    W1bT = lin1_w[:, DM:].T
    sh["w_lin1a"] = np.zeros((128, 128), f32)
    sh["w_lin1b"] = np.zeros((128, 128), f32)
    for b in range(8):
        r = slice(16 * b, 16 * b + 12)
        sh["w_pc"][r, r] = Pc
        sh["w_lin1a"][r, r] = W1aT
        sh["w_lin1b"][r, r] = W1bT
    b1h = lin1_b + lin1_w[:, :DM] @ ffn_ln_b
    b2p = lin2_b - lin2_w.sum(axis=1)
    sh["w_lin2"] = np.zeros((2, 128, 128), f32)
    sh["w_lin3"] = np.zeros((2, 128, 4), f32)
    for g in range(2):
        for bi in range(4):
            b = 4 * g + bi
            sh["w_lin2"][g, 16 * b:16 * b + 12,
                         32 * bi:32 * bi + 20] = lin2_w.T
            sh["w_lin3"][g, 32 * bi:32 * bi + 20, bi] = lin3_w[0]
    sh["w_cnn"] = np.zeros((3, 96, 128), f16)
    for k in range(3):
        for b in range(8):
            sh["w_cnn"][k, 12 * b:12 * b + 12,
                        16 * b:16 * b + 12] = cnn_w[:, :, k].T.astype(f16)
    # scan masks and A scales
    sh["w_mask"] = np.zeros((3, 128, 32), np.float32)
    sh["sc_negA"] = np.zeros((3, 128, 1), f32)
    Asc = -np.exp(A_log)                                       # [24,16]
    for t in range(3):
        for dl in range(8):
            for n in range(DS):
                sh["w_mask"][t, 16 * dl + n, 8 * t + dl] = 1.0
                sh["sc_negA"][t, 16 * dl + n, 0] = Asc[8 * t + dl, n]
    sh["ident"] = np.eye(128, dtype=f16)
    sh["identj"] = np.eye(128, dtype=f16)[::-1].copy()

    def pack(v, blk, nblk):
        o = np.zeros(128, f32)
        for i in range(nblk):
            o[blk * i:blk * i + len(v)] = v
        return o

    vecs = np.zeros((128, 11), f32)
    bconv64 = np.zeros(64, f32)
    bconv64[0:24] = conv_b
    bconv64[32:56] = conv_b
    vecs[:, 0] = np.concatenate([bconv64, bconv64])
    vecs[:, 1] = pack(dt_b, 32, 4)
    vecs[:, 2] = pack(Dp, 32, 4)
    vecs[:, 3] = pack(ln1_g, 16, 8)
    vecs[:, 4] = pack(ln1_b, 16, 8)
    vecs[:, 5] = pack(b1p, 64, 2)
    vecs[:, 6] = pack(ffn_b2, 16, 8)
    vecs[:, 7] = pack(b1h, 16, 8)
    vecs[:, 8] = pack(b2p, 32, 4)
    vecs[:, 9] = pack(cnn_b, 16, 8)
    vecs[:, 10] = pack(b1p * f32(SQ2I), 64, 2)
    sh["vecs"] = vecs
    sh["w_dp"] = np.zeros((4, 128, 32), f32)
    for bi in range(4):
        for c in range(DI):
            sh["w_dp"][bi, 32 * bi + c, c] = Dp[c]
    sh["b_out"] = np.full((8, 1), lin3_b[0], f32)
    sh["b_eps"] = np.full((8, 1), 1e-12, f32)
    # DFT matrices, tiled [mt, cs, kt, 128, 128]
    t_ = np.arange(L, dtype=np.float64)
    f_ = np.arange(NF, dtype=np.float64)
    ang = (2 * np.pi / L) * np.outer(f_, t_)
    wc = np.cos(ang)
    wsn = np.sin(ang)
    wc[1025:] = 0.0
    wsn[1025:] = 0.0
    wdft = np.zeros((NMT, 2, 128, NKT * 128), f16)
    for mt in range(NMT):
        for kt in range(NKT):
            blkc = wc[128 * mt:128 * mt + 128, 128 * kt:128 * kt + 128]
            blks = wsn[128 * mt:128 * mt + 128, 128 * kt:128 * kt + 128]
            wdft[mt, 0, :, 128 * kt:128 * kt + 128] = blkc.T.astype(f16)
            wdft[mt, 1, :, 128 * kt:128 * kt + 128] = blks.T.astype(f16)
    sh["wdft"] = wdft

    # per-core data
    per_core = []
    for c in range(NCORES):
        xl = x[BL * c:BL * c + BL]                             # [8,2048,12]
        xs = np.zeros((4, 96, N), f32)
        for j in range(4):
            for b2 in range(2):
                xb = xl[2 * j + b2]                            # [2048,12]
                for k in range(4):
                    shf = 3 - k
                    r0 = 48 * b2 + 12 * k
                    if shf == 0:
                        xs[j, r0:r0 + 12, :] = xb.T
                    else:
                        xs[j, r0:r0 + 12, shf:] = xb[:-shf].T
        xt = np.zeros((128, NKT * 96), f16)
        for kt in range(NKT):
            xt[:, 96 * kt:96 * kt + 96] = \
                xl[:, 128 * kt:128 * kt + 128].transpose(1, 0, 2) \
                .reshape(128, 96).astype(f16)
        import ml_dtypes as _md
        per_core.append({"xs": xs.astype(_md.bfloat16), "xt": xt})
    return sh, per_core


def kernel(**inputs):
    import ml_dtypes
    sh, per_core = _host_prep(inputs)
    if "nc" not in _CACHE:
        _CACHE["nc"] = _build_module()
    nc = _CACHE["nc"]
    sh = dict(sh)
    for k in ("w_mask", "w_xc", "w_z", "w_delta", "w_bc", "w_dp"):
        sh[k] = sh[k].astype(ml_dtypes.bfloat16)
    in_maps = [{**sh, **pc} for pc in per_core]
    from concourse.bass_utils import run_bass_kernel_spmd
    res = run_bass_kernel_spmd(nc, in_maps, core_ids=list(range(NCORES)))
    outs = [res.results[c]["out"].reshape(BL) for c in range(NCORES)]
    return np.concatenate(outs).astype(np.float32)



# revision 5
# speedup vs baseline: 1.7628x; 1.7628x over previous
"""Trainium2 Bass kernel for nn_Net_90331752170289 (Mamba block + FFT/CNN + fusion head).

Strategy: pure data parallelism over batch (8 batches per core on 8 cores).
Per-core layout: partitions carry (batch, channel) blocks padded to the
32-partition grid, free dim = time (2048 per batch).  The Mamba selective
scan runs as hardware tensor_tensor_scan instructions over (d,n)-partition
tiles in bf16; the FFT is a DFT matmul in fp16 against host-built cos/sin
matrices (half spectrum + mirror via reversed-identity PE transposes); all
small linear layers are block-diagonal float32r matmuls with LN/conv/affine
folds precomputed on the host.

v2: the delta broadcast for the scan is generated on the tensor engine
(selection matmul into PSUM, consumed directly by the ACT exp) instead of
DMA replication — the v1 DMA-broadcast scheme moved ~31MB SBUF->SBUF and
saturated all 8 DMA engines for ~80% of the kernel.  dx/B/C broadcasts stay
on DMA (~20MB).  hc/dBx are single full-tile vector ops, phase A/D psum
tiles are packed to 128 partitions, and the FFN gelu uses the exact-gelu
ACT table directly (table-set-batched phase D) instead of an erf composition.

Block layouts (per core, 8 local batches b, groups g=b//4, bi=b%4):
  X24 tensors (xi, siluz, delta, dx, y): [128, 2048] per g, row = 32*bi + ch
  BC: [128, 2048] per g, row = 32*bi + r (r<16 B, else C)
  X12 tensors (hhat, h_aff, s_t, xm_hat, xcnn): [128, 2048], row = 16*b + m
  scan tiles: [128, 2048] per (b, dn-tile), row = 16*dl + n, d = 8*tile + dl
"""
import numpy as np

B, L, DM = 64, 2048, 12
DI, DS, DC = 24, 16, 4
NCORES = 8
BL = B // NCORES          # 8 local batches per core
N = L                     # free dim per batch
NCH = 1024                # psum chunk (2 banks)
NCY = 512                 # scan-output psum chunk
NF = 1152                 # padded rfft bins (valid 0..1024)
NMT = NF // 128           # 9 DFT M-tiles
NKT = L // 128            # 16 DFT K-tiles

_CACHE = {}


# ---------------------------------------------------------------- device code
def _build_module():
    import concourse.bacc as bacc
    import concourse.bass as bass
    import concourse.tile as tile
    from concourse import mybir
    from contextlib import ExitStack

    F32 = mybir.dt.float32
    F32R = mybir.dt.float32r
    F16 = mybir.dt.float16
    BF16 = mybir.dt.bfloat16
    AF = mybir.ActivationFunctionType
    OP = mybir.AluOpType
    AX = mybir.AxisListType

    nc = bacc.Bacc("TRN2", target_bir_lowering=False, debug=False)

    def din(name, shape, dt=F32R):
        return nc.dram_tensor(name, shape, dt, kind="ExternalInput")

    # per-core data
    xs_d = din("xs", [4, 96, N], BF16)                  # in_proj rhs, per b-pair
    xt_d = din("xt", [128, NKT * 96], F16)        # DFT rhs, kt-major cols
    wdft_d = din("wdft", [NMT, 2, 128, NKT * 128], F16)
    # folded weights (identical on all cores)
    w_xc_d = din("w_xc", [96, 64], BF16)
    w_z_d = din("w_z", [96, 64], BF16)
    w_delta_d = din("w_delta", [128, 128], BF16)
    w_bc_d = din("w_bc", [128, 128], BF16)
    w_sel_d = din("w_sel", [4, 3, 128, 128], BF16)
    w_op_d = din("w_op", [128, 64], BF16)
    w_ones12_d = din("w_ones12", [128, 8], BF16)
    w_bc8_d = din("w_bc8", [8, 128], BF16)
    w_ffn1_d = din("w_ffn1", [4, 128, 128], BF16)
    w_ffn2_d = din("w_ffn2", [4, 128, 32], BF16)
    w_pc_d = din("w_pc", [128, 128], BF16)
    w_lin1a_d = din("w_lin1a", [128, 128], BF16)
    w_lin1b_d = din("w_lin1b", [128, 128], BF16)
    w_lin2_d = din("w_lin2", [2, 128, 128], BF16)
    w_lin3_d = din("w_lin3", [2, 128, 4], BF16)
    w_cnn_d = din("w_cnn", [3, 96, 128], F16)
    w_mask_d = din("w_mask", [3, 128, 32], BF16)
    sc_negA_d = din("sc_negA", [3, 128, 1], F32)
    ident_d = din("ident", [128, 128], F16)
    identj_d = din("identj", [128, 128], F16)
    vec_d = din("vecs", [128, 11], F32)           # packed per-partition vectors
    w_dp_d = din("w_dp", [4, 128, 32], BF16)
    b_out_d = din("b_out", [8, 1], F32)
    b_eps_d = din("b_eps", [8, 1], F32)
    (V_BCONV, V_BDT, V_SDP, V_G1, V_B1, V_BFFN1, V_BFFN2, V_BHEAD1,
     V_BLIN2, V_BCNN, V_BFFN1S) = range(11)

    out_d = nc.dram_tensor("out", [8, 1], F32, kind="ExternalOutput")

    with tile.TileContext(nc) as tc, ExitStack() as ctx:
        sg = ctx.enter_context(tc.tile_pool(name="singles", bufs=1))
        ws = ctx.enter_context(tc.tile_pool(name="work", bufs=2))
        big = ctx.enter_context(tc.tile_pool(name="big", bufs=1))
        pmm = ctx.enter_context(tc.tile_pool(name="pmm", bufs=2, space="PSUM"))
        py = ctx.enter_context(tc.tile_pool(name="py", bufs=2, space="PSUM"))
        pdft = ctx.enter_context(tc.tile_pool(name="pdft", bufs=2, space="PSUM"))

        def load(dram_ap, shape, dt, tag, pool=sg):
            t = pool.tile(shape, dt, tag=tag, name=tag)
            nc.sync.dma_start(out=t, in_=dram_ap)
            return t

        def act(out, in_, func, **kw):
            return nc.scalar.activation(out, in_, func, **kw)

        def mm512(p, lhsT, rhs, c0, c1, start=True, stop=True):
            # matmul into a [*, c1-c0] psum tile in 512-wide pieces
            for o in range(0, c1 - c0, 512):
                w = min(512, c1 - c0 - o)
                nc.tensor.matmul(p[:, o:o + w], lhsT, rhs[:, c0 + o:c0 + o + w],
                                 start=start, stop=stop)

        # ---- load weights/constants into SBUF
        W_xc = load(w_xc_d[:, :], [96, 64], BF16, "w_xc")
        W_z = load(w_z_d[:, :], [96, 64], BF16, "w_z")
        W_delta = load(w_delta_d[:, :], [128, 128], BF16, "w_delta")
        W_bc = load(w_bc_d[:, :], [128, 128], BF16, "w_bc")
        W_sel = [[load(w_sel_d[bi, t], [128, 128], BF16, f"w_sel{bi}{t}")
                  for t in range(3)] for bi in range(4)]
        W_op = load(w_op_d[:, :], [128, 64], BF16, "w_op")
        W_ones12 = load(w_ones12_d[:, :], [128, 8], BF16, "w_ones12")
        W_bc8 = load(w_bc8_d[:, :], [8, 128], BF16, "w_bc8")
        W_ffn1 = [load(w_ffn1_d[q], [128, 128], BF16, f"w_ffn1_{q}")
                  for q in range(4)]
        W_ffn2 = [load(w_ffn2_d[q], [128, 32], BF16, f"w_ffn2_{q}")
                  for q in range(4)]
        W_pc = load(w_pc_d[:, :], [128, 128], BF16, "w_pc")
        W_lin1a = load(w_lin1a_d[:, :], [128, 128], BF16, "w_lin1a")
        W_lin1b = load(w_lin1b_d[:, :], [128, 128], BF16, "w_lin1b")
        W_lin2 = [load(w_lin2_d[g], [128, 128], BF16, f"w_lin2_{g}")
                  for g in range(2)]
        W_lin3 = [load(w_lin3_d[g], [128, 4], BF16, f"w_lin3_{g}")
                  for g in range(2)]
        W_cnn = [load(w_cnn_d[k], [96, 128], F16, f"w_cnn_{k}")
                 for k in range(3)]
        W_mask = [load(w_mask_d[t], [128, 32], BF16, f"w_mask_{t}")
                  for t in range(3)]
        ScA = [load(sc_negA_d[t], [128, 1], F32, f"scA_{t}") for t in range(3)]
        Ident = load(ident_d[:, :], [128, 128], F16, "ident")
        IdentJ = load(identj_d[:, :], [128, 128], F16, "identj")
        Vec_t = load(vec_d[:, :], [128, 11], F32, "vec_t")
        Vec = [Vec_t[:, i:i + 1] for i in range(11)]
        W_dp = [load(w_dp_d[bi], [128, 32], BF16, f"w_dp{bi}")
                for bi in range(4)]
        Bout = load(b_out_d[:, :], [8, 1], F32, "b_out")
        Beps = load(b_eps_d[:, :], [8, 1], F32, "b_eps")

        # ---- persistent activations
        xi = [big.tile([128, N], BF16, tag=f"xi{g}", name=f"xi{g}")
              for g in range(2)]
        siluz = [big.tile([128, N], BF16, tag=f"siluz{g}", name=f"siluz{g}")
                 for g in range(2)]
        delta = [big.tile([128, N], BF16, tag=f"delta{g}", name=f"delta{g}")
                 for g in range(2)]
        dx = [big.tile([128, N], BF16, tag=f"dx{g}", name=f"dx{g}")
              for g in range(2)]
        BC = [big.tile([128, N], BF16, tag=f"bc{g}", name=f"bc{g}")
              for g in range(2)]
        y = [big.tile([128, N], BF16, tag=f"y{g}", name=f"y{g}")
             for g in range(2)]
        xcnn = big.tile([128, N], BF16, tag="xcnn", name="xcnn")
        xfT = big.tile([96, N + 2], F16, tag="xfT", name="xfT")

        CH = [(c * NCH, (c + 1) * NCH) for c in range(N // NCH)]

        # ================= phase A: fused in_proj + causal conv, silu =======
        for g in range(2):
            xsp = [ws.tile([96, N], BF16, tag="pairA", name="xsp")
                   for j in range(2)]
            for j in range(2):
                nc.sync.dma_start(out=xsp[j], in_=xs_d[2 * g + j])
            for c0, c1 in CH:
                p_xc = pmm.tile([128, NCH], F32, tag="pmm", name="pmm")
                p_z = pmm.tile([128, NCH], F32, tag="pmm", name="pmm")
                for j in range(2):
                    jj = slice(64 * j, 64 * j + 64)
                    mm512(p_xc[jj, :], W_xc, xsp[j], c0, c1)
                    mm512(p_z[jj, :], W_z, xsp[j], c0, c1)
                act(xi[g][:, c0:c1], p_xc, AF.Silu, bias=Vec[V_BCONV])
                act(siluz[g][:, c0:c1], p_z, AF.Silu)

        # ================= phase B: x_proj (delta folded), dx ==============
        for g in range(2):
            for c0, c1 in CH:
                p_d = pmm.tile([128, NCH], F32, tag="pmm", name="pmm")
                mm512(p_d, W_delta, xi[g], c0, c1)
                edt = ws.tile([128, NCH], F32, tag="sgm", name="edt")
                act(edt, p_d, AF.Exp, bias=Vec[V_BDT])
                act(delta[g][:, c0:c1], edt, AF.Ln, bias=1.0)
                p_bc = pmm.tile([128, NCH], F32, tag="pmm", name="pmm")
                mm512(p_bc, W_bc, xi[g], c0, c1)
                act(BC[g][:, c0:c1], p_bc, AF.Copy)
            nc.vector.tensor_mul(dx[g], delta[g], xi[g])

        # ================= phase C: selective scan ==========================
        for b in range(BL):
            g, bi = b // 4, b % 4
            # B/C broadcast: tile the per-b [16,N] rows 8x across partitions
            Bbc = ws.tile([128, N], BF16, tag="Bbc", name="Bbc")
            Cbc = ws.tile([128, N], BF16, tag="Cbc", name="Cbc")
            nc.gpsimd.dma_start(out=Bbc[0:16, :],
                                in_=BC[g][32 * bi:32 * bi + 16, :])
            nc.gpsimd.dma_start(out=Cbc[0:16, :],
                                in_=BC[g][32 * bi + 16:32 * bi + 32, :])
            for r in (16, 32, 64):
                nc.gpsimd.dma_start(out=Bbc[r:2 * r, :], in_=Bbc[0:r, :])
                nc.gpsimd.dma_start(out=Cbc[r:2 * r, :], in_=Cbc[0:r, :])
            hcs = []
            for t in range(3):
                r0 = 32 * bi + 8 * t
                # delta broadcast on the PE: selection matmul into PSUM,
                # then exp(A*delta) straight out of PSUM into SBUF bf16.
                a_t = ws.tile([128, N], BF16, tag="a_t", name="a_t")
                for c0, c1 in CH:
                    p_db = pmm.tile([128, NCH], F32, tag="pmm", name="pmm")
                    mm512(p_db, W_sel[bi][t], delta[g], c0, c1)
                    act(a_t[:, c0:c1], p_db, AF.Exp, scale=ScA[t])
                # dx broadcast via one replication DMA
                dx_sl = dx[g][r0:r0 + 8, :]
                dxbc = ws.tile([128, N], BF16, tag="dxbc", name="dxbc")
                nc.sync.dma_start(
                    out=dxbc,
                    in_=bass.AP(tensor=dx_sl.tensor, offset=dx_sl.offset,
                                ap=[dx_sl.ap[0], [0, 16], dx_sl.ap[1]]))
                dBx = ws.tile([128, N], BF16, tag="dBx", name="dBx")
                nc.vector.tensor_mul(dBx, dxbc, Bbc)
                h_t = ws.tile([128, N], BF16, tag="h_t", name="h_t")
                nc.vector.tensor_tensor_scan(h_t, a_t, dBx, 0.0,
                                             OP.mult, OP.add)
                hc = ws.tile([128, N], BF16, tag=f"hc{t}", name="hc", bufs=1)
                nc.vector.tensor_mul(hc, h_t, Cbc)
                hcs.append(hc)
            # y = (ys + xi*Dp) * silu(z)
            rr = slice(32 * bi, 32 * bi + 32)
            for c0 in range(0, N, NCY):
                c1 = c0 + NCY
                p_yt = py.tile([32, NCY], F32, tag="pyt", name="pyt")
                for t in range(3):
                    nc.tensor.matmul(p_yt, W_mask[t], hcs[t][:, c0:c1],
                                     start=(t == 0), stop=False)
                nc.tensor.matmul(p_yt, W_dp[bi], xi[g][:, c0:c1],
                                 start=False, stop=True)
                nc.vector.tensor_mul(y[g][rr, c0:c1], p_yt,
                                     siluz[g][rr, c0:c1])

        # ================= phase E: DFT |FFT| + CNN =========================
        xt_sb = sg.tile([128, NKT * 96], F16, tag="xt", name="xt")
        nc.sync.dma_start(out=xt_sb, in_=xt_d[:, :])
        xf = [sg.tile([128, 96], F16, tag=f"xf{m}", name=f"xf{m}")
              for m in range(NMT)]
        for mt in range(NMT):
            mags = []
            for cs in range(2):
                p_acc = pdft.tile([128, 96], F32, tag="pdft", name="pdft")
                wsl = ws.tile([128, NKT * 128], F16, tag="wsl", name="wsl")
                for hh in range(2):
                    nc.sync.dma_start(
                        out=wsl[:, 1024 * hh:1024 * hh + 1024],
                        in_=wdft_d[mt, cs, :, 1024 * hh:1024 * hh + 1024])
                for kt in range(NKT):
                    nc.tensor.matmul(p_acc, wsl[:, 128 * kt:128 * kt + 128],
                                     xt_sb[:, 96 * kt:96 * kt + 96],
                                     start=(kt == 0), stop=(kt == NKT - 1))
                m_sq = ws.tile([128, 96], F32, tag=f"m_sq{cs}", name="m_sq")
                act(m_sq, p_acc, AF.Square)
                mags.append(m_sq)
            nc.vector.scalar_tensor_tensor(mags[0], mags[0], 1e-20,
                                           mags[1], OP.add, OP.add)
            lnm = ws.tile([128, 96], F32, tag="m_sq1", name="lnm")
            act(lnm, mags[0], AF.Ln)
            act(xf[mt], lnm, AF.Exp, scale=0.5)
        # transpose + mirror into xfT [96, 2+N]: col 1+t = xf[t], cols 0/2049 0
        nc.vector.memset(xfT[:, 0:1], 0.0)
        for mt in range(NMT):
            p_t = pdft.tile([96, 128], F16, tag="pdft", name="pdft")
            nc.tensor.transpose(p_t, xf[mt], Ident)
            act(xfT[:, 1 + 128 * mt:1 + 128 * mt + 128], p_t, AF.Copy)
        for mt in range(8):        # mirrored half: t = 2048 - f, f=128*mt+j
            p_r = pdft.tile([96, 128], F16, tag="pdft", name="pdft")
            nc.tensor.transpose(p_r, xf[mt], IdentJ)
            act(xfT[:, 1922 - 128 * mt:1922 - 128 * mt + 128], p_r, AF.Copy)
        nc.vector.memset(xfT[:, N + 1:N + 2], 0.0)
        # CNN: 3 shifted block-diag matmuls
        for c0, c1 in CH:
            p_cn = pmm.tile([128, NCH], F32, tag="pmm", name="pmm")
            for k in range(3):
                mm512(p_cn, W_cnn[k], xfT, c0 + k, c1 + k,
                      start=(k == 0), stop=(k == 2))
            act(xcnn[:, c0:c1], p_cn, AF.Identity, bias=Vec[V_BCNN])

        # ================= phase D: out_proj -> LN1 -> FFN -> LN2 ==========
        hhat = big.tile([128, N], BF16, tag="hhat", name="hhat")
        h_aff = big.tile([128, N], BF16, tag="h_aff", name="h_aff")
        for c0, c1 in CH:
            p_m = pmm.tile([128, NCH], F32, tag="pmm", name="pmm")
            for g in range(2):
                mm512(p_m[64 * g:64 * g + 64, :], W_op, y[g], c0, c1)
            cent = ws.tile([128, NCH], BF16, tag="cent", name="cent")
            sq = ws.tile([128, NCH], BF16, tag="sq", name="sq")
            act(cent, p_m, AF.Copy)
            act(sq, p_m, AF.Square)
            p_v = pmm.tile([8, NCH], F32, tag="pmm", name="pmm")
            mm512(p_v, W_ones12, sq, 0, NCH)
            sd = ws.tile([8, NCH], BF16, tag="sd", name="sd")
            act(sd, p_v, AF.Ln, bias=Beps)
            inv = ws.tile([8, NCH], BF16, tag="sd", name="inv")
            act(inv, sd, AF.Exp, scale=-0.5)
            p_b = pmm.tile([128, NCH], F32, tag="pmm", name="pmm")
            mm512(p_b, W_bc8, inv, 0, NCH)
            nc.vector.tensor_mul(hhat[:, c0:c1], cent, p_b)
            nc.vector.tensor_scalar(h_aff[:, c0:c1], hhat[:, c0:c1],
                                    Vec[V_G1], Vec[V_B1], OP.mult, OP.add)
        # FFN: gelu via the exact-gelu ACT table (all gelu calls batched)
        s_t = big.tile([128, N], BF16, tag="s_t", name="s_t")
        for q in range(4):
            for c0, c1 in CH:
                p_f = pmm.tile([128, NCH], F32, tag="pmm", name="pmm")
                mm512(p_f, W_ffn1[q], hhat, c0, c1)
                ff_c = ws.tile([128, NCH], BF16, tag="ffch", name="ff_c")
                act(ff_c, p_f, AF.Gelu, bias=Vec[V_BFFN1])
                p_2 = pmm.tile([32, NCH], F32, tag="pmm", name="pmm")
                mm512(p_2, W_ffn2[q], ff_c, 0, NCH)
                rq = slice(32 * q, 32 * q + 32)
                nc.vector.scalar_tensor_tensor(
                    s_t[rq, c0:c1], p_2, Vec[V_BFFN2][rq, :],
                    h_aff[rq, c0:c1], OP.add, OP.add)
        # LN2
        xm_hat = big.tile([128, N], BF16, tag="xm_hat", name="xm_hat")
        for c0, c1 in CH:
            p_c = pmm.tile([128, NCH], F32, tag="pmm", name="pmm")
            mm512(p_c, W_pc, s_t, c0, c1)
            c2 = ws.tile([128, NCH], BF16, tag="cent", name="c2")
            act(c2, p_c, AF.Copy)
            sq2 = ws.tile([128, NCH], BF16, tag="sq", name="sq2")
            act(sq2, p_c, AF.Square)
            p_v2 = pmm.tile([8, NCH], F32, tag="pmm", name="pmm")
            mm512(p_v2, W_ones12, sq2, 0, NCH)
            sd2 = ws.tile([8, NCH], BF16, tag="sd", name="sd2")
            act(sd2, p_v2, AF.Ln, bias=Beps)
            inv2 = ws.tile([8, NCH], BF16, tag="sd", name="inv2")
            act(inv2, sd2, AF.Exp, scale=-0.5)
            p_b2 = pmm.tile([128, NCH], F32, tag="pmm", name="pmm")
            mm512(p_b2, W_bc8, inv2, 0, NCH)
            nc.vector.tensor_mul(xm_hat[:, c0:c1], c2, p_b2)

        # ================= phase F: fusion head =============================
        racc = [sg.tile([4, 1], F32, tag=f"racc{g}", name=f"racc{g}")
                for g in range(2)]
        for g in range(2):
            nc.vector.memset(racc[g], 0.0)
        for c0, c1 in CH:
            p_1 = pmm.tile([128, NCH], F32, tag="pmm", name="pmm")
            mm512(p_1, W_lin1a, xm_hat, c0, c1, start=True, stop=False)
            mm512(p_1, W_lin1b, xcnn, c0, c1, start=False, stop=True)
            mneg = ws.tile([128, NCH], BF16, tag="mneg", name="mneg")
            nc.vector.tensor_scalar(mneg, p_1, Vec[V_BHEAD1], 0.0,
                                    OP.add, OP.min)
            e_t = ws.tile([128, NCH], BF16, tag="e_t", name="e_t")
            act(e_t, mneg, AF.Exp)
            r_t = ws.tile([128, NCH], BF16, tag="mneg", name="r_t")
            act(r_t, p_1, AF.Relu, bias=Vec[V_BHEAD1])
            v_t = ws.tile([128, NCH], BF16, tag="e_t", name="v_t")
            nc.vector.tensor_add(v_t, r_t, e_t)
            for g in range(2):
                p_o2 = pmm.tile([128, NCH], F32, tag="pmm", name="pmm")
                mm512(p_o2, W_lin2[g], v_t, 0, NCH)
                o2c = ws.tile([128, NCH], BF16, tag="mneg", name="o2c")
                act(o2c, p_o2, AF.Identity, bias=Vec[V_BLIN2])
                p_o3 = pmm.tile([4, NCH], F32, tag="pmm", name="pmm")
                mm512(p_o3, W_lin3[g], o2c, 0, NCH)
                o3c = ws.tile([4, NCH], F32, tag="o3c", name="o3c")
                act(o3c, p_o3, AF.Copy)
                rc = ws.tile([4, 1], F32, tag="rc", name="rc")
                nc.vector.tensor_reduce(rc, o3c, AX.X, OP.add)
                nc.vector.tensor_add(racc[g], racc[g], rc)
        for g in range(2):
            res = sg.tile([4, 1], F32, tag=f"res{g}", name=f"res{g}")
            act(res, racc[g], AF.Sigmoid, bias=Bout[0:4, :], scale=1.0 / N)
            nc.sync.dma_start(out=out_d[4 * g:4 * g + 4, :], in_=res)

    # Prefer the combined ln+exp ACT table: hide Exp/Ln from all other
    # tables so the table-load pass lands on natural_log_exp_and_others
    # (availability-only metadata; claiming less than reality is safe).
    import concourse.bacc as bacc_mod
    from concourse import mybir as _mb
    _orig_gat = bacc_mod.get_activation_tables

    def _gat(arch):
        t = {k: set(v) for k, v in _orig_gat(arch).items()}
        for name, s in t.items():
            if name != "natural_log_exp_and_others":
                s.discard(_mb.ActivationFunctionType.Exp)
                s.discard(_mb.ActivationFunctionType.Ln)
        return t

    bacc_mod.get_activation_tables = _gat
    try:
        nc.compile()
    finally:
        bacc_mod.get_activation_tables = _orig_gat
    return nc


# ---------------------------------------------------------------- host side
def _host_prep(inputs):
    f32, f16 = np.float32, np.float16
    x = inputs["x"].astype(f32)
    in_proj_w = inputs["in_proj_w"].astype(f32)
    conv_w = inputs["conv_w"].astype(f32)
    conv_b = inputs["conv_b"].astype(f32)
    x_proj_w = inputs["x_proj_w"].astype(f32)
    dt_w = inputs["dt_w"].astype(f32)
    dt_b = inputs["dt_b"].astype(f32)
    A_log = inputs["A_log"].astype(f32)
    Dp = inputs["Dp"].astype(f32)
    out_proj_w = inputs["out_proj_w"].astype(f32)
    ln1_g, ln1_b = inputs["ln1_g"].astype(f32), inputs["ln1_b"].astype(f32)
    ffn_w1, ffn_b1 = inputs["ffn_w1"].astype(f32), inputs["ffn_b1"].astype(f32)
    ffn_w2, ffn_b2 = inputs["ffn_w2"].astype(f32), inputs["ffn_b2"].astype(f32)
    ffn_ln_g = inputs["ffn_ln_g"].astype(f32)
    ffn_ln_b = inputs["ffn_ln_b"].astype(f32)
    cnn_w, cnn_b = inputs["cnn_w"].astype(f32), inputs["cnn_b"].astype(f32)
    lin1_w, lin1_b = inputs["lin1_w"].astype(f32), inputs["lin1_b"].astype(f32)
    lin2_w, lin2_b = inputs["lin2_w"].astype(f32), inputs["lin2_b"].astype(f32)
    lin3_w, lin3_b = inputs["lin3_w"].astype(f32), inputs["lin3_b"].astype(f32)

    sh = {}
    # fused in_proj + conv:  Wxc[k*12+m, d] = conv_w[d,0,k]*in_proj_w[d,m]
    Wxc = np.einsum('dk,dm->kmd', conv_w[:, 0, :], in_proj_w[:DI]).reshape(48, DI)
    sh["w_xc"] = np.zeros((96, 64), f32)
    sh["w_z"] = np.zeros((96, 64), f32)
    for b2 in range(2):
        sh["w_xc"][48 * b2:48 * b2 + 48, 32 * b2:32 * b2 + 24] = Wxc
        for m in range(DM):
            sh["w_z"][48 * b2 + 36 + m, 32 * b2:32 * b2 + 24] = in_proj_w[DI:, m]
    # x_proj (delta rank-1 folded)
    Wdelta = np.einsum('d,j->jd', dt_w[:, 0], x_proj_w[0])     # [24,24]
    WBC = x_proj_w[1:].T                                       # [24,32]
    sh["w_delta"] = np.zeros((128, 128), f32)
    sh["w_bc"] = np.zeros((128, 128), f32)
    for bi in range(4):
        r = slice(32 * bi, 32 * bi + 24)
        sh["w_delta"][r, 32 * bi:32 * bi + 24] = Wdelta
        sh["w_bc"][r, 32 * bi:32 * bi + 32] = WBC
    # delta broadcast selection: out row 16*dl+n <- delta row 32*bi+8*t+dl
    sh["w_sel"] = np.zeros((4, 3, 128, 128), f32)
    for bi in range(4):
        for t in range(3):
            for dl in range(8):
                for n in range(DS):
                    sh["w_sel"][bi, t, 32 * bi + 8 * t + dl, 16 * dl + n] = 1.0
    # out_proj with centering fold
    Pc = np.eye(DM, dtype=f32) - f32(1.0 / DM)
    WopT = (Pc @ out_proj_w).T                                 # [24,12]
    sh["w_op"] = np.zeros((128, 64), f32)
    for bi in range(4):
        sh["w_op"][32 * bi:32 * bi + 24, 16 * bi:16 * bi + 12] = WopT
    sh["w_ones12"] = np.zeros((128, 8), f32)
    sh["w_bc8"] = np.zeros((8, 128), f32)
    for b in range(8):
        sh["w_ones12"][16 * b:16 * b + 12, b] = f32(1.0 / DM)
        sh["w_bc8"][b, 16 * b:16 * b + 16] = 1.0
    # ffn
    W1p = (ffn_w1 * ln1_g[None, :]).T                          # [12,48]
    b1p = ffn_b1 + ffn_w1 @ ln1_b
    sh["w_ffn1"] = np.zeros((4, 128, 128), f32)
    sh["w_ffn2"] = np.zeros((4, 128, 32), f32)
    for q in range(4):
        for b2 in range(2):
            b = 2 * q + b2
            sh["w_ffn1"][q, 16 * b:16 * b + 12, 64 * b2:64 * b2 + 48] = W1p
            sh["w_ffn2"][q, 64 * b2:64 * b2 + 48,
                         16 * b2:16 * b2 + 12] = ffn_w2.T
    sh["w_pc"] = np.zeros((128, 128), f32)
    W1aT = (lin1_w[:, :DM] * ffn_ln_g[None, :]).T              # [12,12]
    W1bT = lin1_w[:, DM:].T
    sh["w_lin1a"] = np.zeros((128, 128), f32)
    sh["w_lin1b"] = np.zeros((128, 128), f32)
    for b in range(8):
        r = slice(16 * b, 16 * b + 12)
        sh["w_pc"][r, r] = Pc
        sh["w_lin1a"][r, r] = W1aT
        sh["w_lin1b"][r, r] = W1bT
    b1h = lin1_b + lin1_w[:, :DM] @ ffn_ln_b
    b2p = lin2_b - lin2_w.sum(axis=1)
    sh["w_lin2"] = np.zeros((2, 128, 128), f32)
    sh["w_lin3"] = np.zeros((2, 128, 4), f32)
    for g in range(2):
        for bi in range(4):
            b = 4 * g + bi
            sh["w_lin2"][g, 16 * b:16 * b + 12,
                         32 * bi:32 * bi + 20] = lin2_w.T
            sh["w_lin3"][g, 32 * bi:32 * bi + 20, bi] = lin3_w[0]
    sh["w_cnn"] = np.zeros((3, 96, 128), f16)
    for k in range(3):
        for b in range(8):
            sh["w_cnn"][k, 12 * b:12 * b + 12,
                        16 * b:16 * b + 12] = cnn_w[:, :, k].T.astype(f16)
    # scan masks and A scales
    sh["w_mask"] = np.zeros((3, 128, 32), np.float32)
    sh["sc_negA"] = np.zeros((3, 128, 1), f32)
    Asc = -np.exp(A_log)                                       # [24,16]
    for t in range(3):
        for dl in range(8):
            for n in range(DS):
                sh["w_mask"][t, 16 * dl + n, 8 * t + dl] = 1.0
                sh["sc_negA"][t, 16 * dl + n, 0] = Asc[8 * t + dl, n]
    sh["ident"] = np.eye(128, dtype=f16)
    sh["identj"] = np.eye(128, dtype=f16)[::-1].copy()

    def pack(v, blk, nblk):
        o = np.zeros(128, f32)
        for i in range(nblk):
            o[blk * i:blk * i + len(v)] = v
        return o

    vecs = np.zeros((128, 11), f32)
    bconv64 = np.zeros(64, f32)
    bconv64[0:24] = conv_b
    bconv64[32:56] = conv_b
    vecs[:, 0] = np.concatenate([bconv64, bconv64])
    vecs[:, 1] = pack(dt_b, 32, 4)
    vecs[:, 2] = pack(Dp, 32, 4)
    vecs[:, 3] = pack(ln1_g, 16, 8)
    vecs[:, 4] = pack(ln1_b, 16, 8)
    vecs[:, 5] = pack(b1p, 64, 2)
    vecs[:, 6] = pack(ffn_b2, 16, 8)
    vecs[:, 7] = pack(b1h, 16, 8)
    vecs[:, 8] = pack(b2p, 32, 4)
    vecs[:, 9] = pack(cnn_b, 16, 8)
    vecs[:, 10] = pack(b1p, 64, 2)
    sh["vecs"] = vecs
    sh["w_dp"] = np.zeros((4, 128, 32), f32)
    for bi in range(4):
        for c in range(DI):
            sh["w_dp"][bi, 32 * bi + c, c] = Dp[c]
    sh["b_out"] = np.full((8, 1), lin3_b[0], f32)
    sh["b_eps"] = np.full((8, 1), 1e-12, f32)
    # DFT matrices, tiled [mt, cs, kt, 128, 128]
    t_ = np.arange(L, dtype=np.float64)
    f_ = np.arange(NF, dtype=np.float64)
    ang = (2 * np.pi / L) * np.outer(f_, t_)
    wc = np.cos(ang)
    wsn = np.sin(ang)
    wc[1025:] = 0.0
    wsn[1025:] = 0.0
    wdft = np.zeros((NMT, 2, 128, NKT * 128), f16)
    for mt in range(NMT):
        for kt in range(NKT):
            blkc = wc[128 * mt:128 * mt + 128, 128 * kt:128 * kt + 128]
            blks = wsn[128 * mt:128 * mt + 128, 128 * kt:128 * kt + 128]
            wdft[mt, 0, :, 128 * kt:128 * kt + 128] = blkc.T.astype(f16)
            wdft[mt, 1, :, 128 * kt:128 * kt + 128] = blks.T.astype(f16)
    sh["wdft"] = wdft

    # per-core data
    per_core = []
    for c in range(NCORES):
        xl = x[BL * c:BL * c + BL]                             # [8,2048,12]
        xs = np.zeros((4, 96, N), f32)
        for j in range(4):
            for b2 in range(2):
                xb = xl[2 * j + b2]                            # [2048,12]
                for k in range(4):
                    shf = 3 - k
                    r0 = 48 * b2 + 12 * k
                    if shf == 0:
                        xs[j, r0:r0 + 12, :] = xb.T
                    else:
                        xs[j, r0:r0 + 12, shf:] = xb[:-shf].T
        xt = np.zeros((128, NKT * 96), f16)
        for kt in range(NKT):
            xt[:, 96 * kt:96 * kt + 96] = \
                xl[:, 128 * kt:128 * kt + 128].transpose(1, 0, 2) \
                .reshape(128, 96).astype(f16)
        import ml_dtypes as _md
        per_core.append({"xs": xs.astype(_md.bfloat16), "xt": xt})
    return sh, per_core


def kernel(**inputs):
    import ml_dtypes
    sh, per_core = _host_prep(inputs)
    if "nc" not in _CACHE:
        _CACHE["nc"] = _build_module()
    nc = _CACHE["nc"]
    sh = dict(sh)
    for k in ("w_mask", "w_xc", "w_z", "w_delta", "w_bc", "w_dp", "w_sel",
          "w_op", "w_ones12", "w_bc8", "w_ffn1", "w_ffn2", "w_pc",
          "w_lin1a", "w_lin1b", "w_lin2", "w_lin3"):
        sh[k] = sh[k].astype(ml_dtypes.bfloat16)
    in_maps = [{**sh, **pc} for pc in per_core]
    from concourse.bass_utils import run_bass_kernel_spmd
    res = run_bass_kernel_spmd(nc, in_maps, core_ids=list(range(NCORES)))
    outs = [res.results[c]["out"].reshape(BL) for c in range(NCORES)]
    return np.concatenate(outs).astype(np.float32)


# revision 13
# speedup vs baseline: 2.0647x; 1.1713x over previous
"""Trainium2 Bass kernel for nn_Net_90331752170289 (Mamba block + FFT/CNN + fusion head).

Strategy: pure data parallelism over batch (8 batches per core on 8 cores).
Per-core layout: partitions carry (batch, channel) blocks padded to the
32-partition grid, free dim = time (2048 per batch).  The Mamba selective
scan runs as hardware tensor_tensor_scan instructions over (d,n)-partition
tiles in bf16; the FFT is a DFT matmul in fp16 against host-built cos/sin
matrices (half spectrum + mirror via reversed-identity PE transposes); all
small linear layers are block-diagonal float32r matmuls with LN/conv/affine
folds precomputed on the host.

v2: the delta broadcast for the scan is generated on the tensor engine
(selection matmul into PSUM, consumed directly by the ACT exp) instead of
DMA replication — the v1 DMA-broadcast scheme moved ~31MB SBUF->SBUF and
saturated all 8 DMA engines for ~80% of the kernel.  dx/B/C broadcasts stay
on DMA (~20MB).  hc/dBx are single full-tile vector ops, phase A/D psum
tiles are packed to 128 partitions, and the FFN gelu uses the exact-gelu
ACT table directly (table-set-batched phase D) instead of an erf composition.

Block layouts (per core, 8 local batches b, groups g=b//4, bi=b%4):
  X24 tensors (xi, siluz, delta, dx, y): [128, 2048] per g, row = 32*bi + ch
  BC: [128, 2048] per g, row = 32*bi + r (r<16 B, else C)
  X12 tensors (hhat, h_aff, s_t, xm_hat, xcnn): [128, 2048], row = 16*b + m
  scan tiles: [128, 2048] per (b, dn-tile), row = 16*dl + n, d = 8*tile + dl
"""
import numpy as np

B, L, DM = 64, 2048, 12
DI, DS, DC = 24, 16, 4
NCORES = 8
BL = B // NCORES          # 8 local batches per core
N = L                     # free dim per batch
NCH = 1024                # psum chunk (2 banks)
NCY = 512                 # scan-output psum chunk
NF = 1152                 # padded rfft bins (valid 0..1024)
NMT = NF // 128           # 9 DFT M-tiles
NKT = L // 128            # 16 DFT K-tiles

_CACHE = {}


# ---------------------------------------------------------------- device code
def _build_module():
    import concourse.bacc as bacc
    import concourse.bass as bass
    import concourse.tile as tile
    from concourse import mybir
    from contextlib import ExitStack

    F32 = mybir.dt.float32
    F32R = mybir.dt.float32r
    F16 = mybir.dt.float16
    BF16 = mybir.dt.bfloat16
    AF = mybir.ActivationFunctionType
    OP = mybir.AluOpType
    AX = mybir.AxisListType

    nc = bacc.Bacc("TRN2", target_bir_lowering=False, debug=False)

    def din(name, shape, dt=F32R):
        return nc.dram_tensor(name, shape, dt, kind="ExternalInput")

    # per-core data
    xs_d = din("xs", [4, 96, N], BF16)                  # in_proj rhs, per b-pair
    xt_d = din("xt", [128, NKT * 96], F16)        # DFT rhs, kt-major cols
    wdft_d = din("wdft", [NMT, 2, 128, NKT * 128], F16)
    # folded weights (identical on all cores)
    w_xc_d = din("w_xc", [96, 64], BF16)
    w_z_d = din("w_z", [96, 64], BF16)
    w_delta_d = din("w_delta", [128, 128], BF16)
    w_bc_d = din("w_bc", [128, 128], BF16)
    w_sel_d = din("w_sel", [4, 3, 128, 128], BF16)
    w_selbc_d = din("w_selbc", [4, 2, 128, 128], BF16)
    w_op_d = din("w_op", [128, 64], BF16)
    w_ones12_d = din("w_ones12", [128, 8], BF16)
    w_bc8_d = din("w_bc8", [8, 128], BF16)
    w_ffn1_d = din("w_ffn1", [4, 128, 128], BF16)
    w_ffn2_d = din("w_ffn2", [4, 128, 32], BF16)
    w_pc_d = din("w_pc", [128, 128], BF16)
    w_lin1a_d = din("w_lin1a", [128, 128], BF16)
    w_lin1b_d = din("w_lin1b", [128, 128], BF16)
    w_lin2_d = din("w_lin2", [2, 128, 128], BF16)
    w_lin3_d = din("w_lin3", [2, 128, 4], BF16)
    w_cnn_d = din("w_cnn", [3, 96, 128], F16)
    w_mask_d = din("w_mask", [3, 128, 32], BF16)
    sc_negA_d = din("sc_negA", [3, 128, 1], F32)
    ident_d = din("ident", [128, 128], F16)
    identj_d = din("identj", [128, 128], F16)
    vec_d = din("vecs", [128, 11], F32)           # packed per-partition vectors
    w_dp_d = din("w_dp", [4, 128, 32], BF16)
    b_out_d = din("b_out", [8, 1], F32)
    b_eps_d = din("b_eps", [8, 1], F32)
    (V_BCONV, V_BDT, V_SDP, V_G1, V_B1, V_BFFN1, V_BFFN2, V_BHEAD1,
     V_BLIN2, V_BCNN, V_BFFN1S) = range(11)

    out_d = nc.dram_tensor("out", [8, 1], F32, kind="ExternalOutput")
    DBG = _CACHE.get("dbg", False)
    if DBG:
        dbg_d = {k: nc.dram_tensor(f"dbg_{k}", [128, N], BF16,
                                   kind="ExternalOutput")
                 for k in ("y0", "y1", "hhat", "s_t", "xm_hat", "xcnn")}

    with tile.TileContext(nc) as tc, ExitStack() as ctx:
        sg = ctx.enter_context(tc.tile_pool(name="singles", bufs=1))
        ws = ctx.enter_context(tc.tile_pool(name="work", bufs=2))
        big = ctx.enter_context(tc.tile_pool(name="big", bufs=1))
        pmm = ctx.enter_context(tc.tile_pool(name="pmm", bufs=2, space="PSUM"))
        py = ctx.enter_context(tc.tile_pool(name="py", bufs=2, space="PSUM"))
        pdft = ctx.enter_context(tc.tile_pool(name="pdft", bufs=2, space="PSUM"))

        def load(dram_ap, shape, dt, tag, pool=sg):
            t = pool.tile(shape, dt, tag=tag, name=tag)
            nc.sync.dma_start(out=t, in_=dram_ap)
            return t

        def act(out, in_, func, **kw):
            return nc.scalar.activation(out, in_, func, **kw)

        def mm512(p, lhsT, rhs, c0, c1, start=True, stop=True):
            # matmul into a [*, c1-c0] psum tile in 512-wide pieces
            for o in range(0, c1 - c0, 512):
                w = min(512, c1 - c0 - o)
                nc.tensor.matmul(p[:, o:o + w], lhsT, rhs[:, c0 + o:c0 + o + w],
                                 start=start, stop=stop)

        # ---- load weights/constants into SBUF
        W_xc = load(w_xc_d[:, :], [96, 64], BF16, "w_xc")
        W_z = load(w_z_d[:, :], [96, 64], BF16, "w_z")
        W_delta = load(w_delta_d[:, :], [128, 128], BF16, "w_delta")
        W_bc = load(w_bc_d[:, :], [128, 128], BF16, "w_bc")
        W_sel = [[load(w_sel_d[bi, t], [128, 128], BF16, f"w_sel{bi}{t}")
                  for t in range(3)] for bi in range(4)]
        W_selBC = [[load(w_selbc_d[bi, u], [128, 128], BF16,
                         f"w_selbc{bi}{u}") for u in range(2)]
                   for bi in range(4)]
        W_op = load(w_op_d[:, :], [128, 64], BF16, "w_op")
        W_ones12 = load(w_ones12_d[:, :], [128, 8], BF16, "w_ones12")
        W_bc8 = load(w_bc8_d[:, :], [8, 128], BF16, "w_bc8")
        W_ffn1 = [load(w_ffn1_d[q], [128, 128], BF16, f"w_ffn1_{q}")
                  for q in range(4)]
        W_ffn2 = [load(w_ffn2_d[q], [128, 32], BF16, f"w_ffn2_{q}")
                  for q in range(4)]
        W_pc = load(w_pc_d[:, :], [128, 128], BF16, "w_pc")
        W_lin1a = load(w_lin1a_d[:, :], [128, 128], BF16, "w_lin1a")
        W_lin1b = load(w_lin1b_d[:, :], [128, 128], BF16, "w_lin1b")
        W_lin2 = [load(w_lin2_d[g], [128, 128], BF16, f"w_lin2_{g}")
                  for g in range(2)]
        W_lin3 = [load(w_lin3_d[g], [128, 4], BF16, f"w_lin3_{g}")
                  for g in range(2)]
        W_cnn = [load(w_cnn_d[k], [96, 128], F16, f"w_cnn_{k}")
                 for k in range(3)]
        W_mask = [load(w_mask_d[t], [128, 32], BF16, f"w_mask_{t}")
                  for t in range(3)]
        ScA = [load(sc_negA_d[t], [128, 1], F32, f"scA_{t}") for t in range(3)]
        Ident = load(ident_d[:, :], [128, 128], F16, "ident")
        IdentJ = load(identj_d[:, :], [128, 128], F16, "identj")
        Vec_t = load(vec_d[:, :], [128, 11], F32, "vec_t")
        Vec = [Vec_t[:, i:i + 1] for i in range(11)]
        W_dp = [load(w_dp_d[bi], [128, 32], BF16, f"w_dp{bi}")
                for bi in range(4)]
        Bout = load(b_out_d[:, :], [8, 1], F32, "b_out")
        Beps = load(b_eps_d[:, :], [8, 1], F32, "b_eps")

        # ---- persistent activations
        xi = [big.tile([128, N], BF16, tag=f"xi{g}", name=f"xi{g}")
              for g in range(2)]
        siluz = [big.tile([128, N], BF16, tag=f"siluz{g}", name=f"siluz{g}")
                 for g in range(2)]
        delta = [big.tile([128, N], BF16, tag=f"delta{g}", name=f"delta{g}")
                 for g in range(2)]
        dx = [big.tile([128, N], BF16, tag=f"dx{g}", name=f"dx{g}")
              for g in range(2)]
        BC = [big.tile([128, N], BF16, tag=f"bc{g}", name=f"bc{g}")
              for g in range(2)]
        y = [big.tile([128, N], BF16, tag=f"y{g}", name=f"y{g}")
             for g in range(2)]
        xcnn = big.tile([128, N], BF16, tag="xcnn", name="xcnn")
        xfT = big.tile([96, N + 2], F16, tag="xfT", name="xfT")

        CH = [(c * NCH, (c + 1) * NCH) for c in range(N // NCH)]

        # ================= phase A: fused in_proj + causal conv, silu =======
        for g in range(2):
            xsp = [ws.tile([96, N], BF16, tag="pairA", name="xsp")
                   for j in range(2)]
            for j in range(2):
                nc.sync.dma_start(out=xsp[j], in_=xs_d[2 * g + j])
            for c0, c1 in CH:
                p_xc = pmm.tile([128, NCH], F32, tag="pmm", name="pmm")
                p_z = pmm.tile([128, NCH], F32, tag="pmm", name="pmm")
                for j in range(2):
                    jj = slice(64 * j, 64 * j + 64)
                    mm512(p_xc[jj, :], W_xc, xsp[j], c0, c1)
                    mm512(p_z[jj, :], W_z, xsp[j], c0, c1)
                act(xi[g][:, c0:c1], p_xc, AF.Silu, bias=Vec[V_BCONV])
                act(siluz[g][:, c0:c1], p_z, AF.Silu)

        # ================= phase B: x_proj (delta folded), dx ==============
        for g in range(2):
            for c0, c1 in CH:
                p_d = pmm.tile([128, NCH], F32, tag="pmm", name="pmm")
                mm512(p_d, W_delta, xi[g], c0, c1)
                edt = ws.tile([128, NCH], F32, tag="sgm", name="edt")
                act(edt, p_d, AF.Exp, bias=Vec[V_BDT])
                act(delta[g][:, c0:c1], edt, AF.Ln, bias=1.0)
                p_bc = pmm.tile([128, NCH], F32, tag="pmm", name="pmm")
                mm512(p_bc, W_bc, xi[g], c0, c1)
                act(BC[g][:, c0:c1], p_bc, AF.Copy)
            nc.vector.tensor_mul(dx[g], delta[g], xi[g])

        # ================= phase E setup (DFT |FFT|, interleaved into C) ====
        xt_sb = sg.tile([128, NKT * 96], F16, tag="xt", name="xt")
        nc.sync.dma_start(out=xt_sb, in_=xt_d[:, :])
        xf = [sg.tile([128, 96], F16, tag=f"xf{m}", name=f"xf{m}")
              for m in range(NMT)]
        nc.vector.memset(xfT[:, 0:1], 0.0)

        def e_mag(mt):
            # one DFT M-tile: |FFT| magnitudes + transpose/mirror into xfT
            mags = []
            for cs in range(2):
                p_acc = pdft.tile([128, 96], F32, tag="pdft", name="pdft")
                wsl = ws.tile([128, NKT * 128], F16, tag="wsl", name="wsl")
                for hh in range(2):
                    nc.sync.dma_start(
                        out=wsl[:, 1024 * hh:1024 * hh + 1024],
                        in_=wdft_d[mt, cs, :, 1024 * hh:1024 * hh + 1024])
                for kt in range(NKT):
                    nc.tensor.matmul(p_acc, wsl[:, 128 * kt:128 * kt + 128],
                                     xt_sb[:, 96 * kt:96 * kt + 96],
                                     start=(kt == 0), stop=(kt == NKT - 1))
                m_sq = ws.tile([128, 96], F32, tag=f"m_sq{cs}", name="m_sq")
                act(m_sq, p_acc, AF.Square)
                mags.append(m_sq)
            nc.vector.scalar_tensor_tensor(mags[0], mags[0], 1e-20,
                                           mags[1], OP.add, OP.add)
            lnm = ws.tile([128, 96], F32, tag="m_sq1", name="lnm")
            act(lnm, mags[0], AF.Ln)
            act(xf[mt], lnm, AF.Exp, scale=0.5)
            p_t = pdft.tile([96, 128], F16, tag="pdft", name="pdft")
            nc.tensor.transpose(p_t, xf[mt], Ident)
            if mt < 8:
                act(xfT[:, 1 + 128 * mt:1 + 128 * mt + 128], p_t, AF.Copy)
            else:
                # only f=1024 is a valid forward bin; cols 1026.. belong to
                # the mirrored half written by mt<=7
                act(xfT[:, 1025:1026], p_t[:, 0:1], AF.Copy)
            if mt < 8:             # mirrored half: t = 2048 - f, f=128*mt+j
                p_r = pdft.tile([96, 128], F16, tag="pdft", name="pdft")
                nc.tensor.transpose(p_r, xf[mt], IdentJ)
                act(xfT[:, 1922 - 128 * mt:1922 - 128 * mt + 128],
                    p_r, AF.Copy)
                if mt == 0:
                    # mirror of mt=0 writes through col N+1; re-zero the
                    # conv right-pad column after it
                    nc.vector.memset(xfT[:, N + 1:N + 2], 0.0)

        # ================= phase C: selective scan ==========================
        for b in range(BL):
            g, bi = b // 4, b % 4
            # B/C broadcast: selection matmul on the PE + ACT copy out of
            # PSUM (the v1/v2 DMA doubling chains serialized on the SW DMA
            # queues and stalled the scan ~19us per batch pair).
            Bbc = ws.tile([128, N], BF16, tag="Bbc", name="Bbc")
            Cbc = ws.tile([128, N], BF16, tag="Cbc", name="Cbc")
            for u, dst in ((0, Bbc), (1, Cbc)):
                for c0, c1 in CH:
                    p_bb = pmm.tile([128, NCH], F32, tag="pmm", name="pmm")
                    mm512(p_bb, W_selBC[bi][u], BC[g], c0, c1)
                    act(dst[:, c0:c1], p_bb, AF.Copy)
            hcs = []
            for t in range(3):
                r0 = 32 * bi + 8 * t
                # delta broadcast on the PE: selection matmul into PSUM,
                # then exp(A*delta) straight out of PSUM into SBUF bf16.
                a_t = ws.tile([128, N], BF16, tag="a_t", name="a_t")
                for c0, c1 in CH:
                    p_db = pmm.tile([128, NCH], F32, tag="pmm", name="pmm")
                    mm512(p_db, W_sel[bi][t], delta[g], c0, c1)
                    act(a_t[:, c0:c1], p_db, AF.Exp, scale=ScA[t])
                # dx broadcast via one replication DMA
                dx_sl = dx[g][r0:r0 + 8, :]
                dxbc = ws.tile([128, N], BF16, tag="dxbc", name="dxbc")
                nc.sync.dma_start(
                    out=dxbc,
                    in_=bass.AP(tensor=dx_sl.tensor, offset=dx_sl.offset,
                                ap=[dx_sl.ap[0], [0, 16], dx_sl.ap[1]]))
                dBx = ws.tile([128, N], BF16, tag="dBx", name="dBx")
                nc.vector.tensor_mul(dBx, dxbc, Bbc)
                h_t = ws.tile([128, N], BF16, tag="h_t", name="h_t")
                nc.vector.tensor_tensor_scan(h_t, a_t, dBx, 0.0,
                                             OP.mult, OP.add)
                hc = ws.tile([128, N], BF16, tag=f"hc{t}", name="hc", bufs=1)
                nc.vector.tensor_mul(hc, h_t, Cbc)
                hcs.append(hc)
            # y = (ys + xi*Dp) * silu(z)
            rr = slice(32 * bi, 32 * bi + 32)
            for c0 in range(0, N, NCY):
                c1 = c0 + NCY
                p_yt = py.tile([32, NCY], F32, tag="pyt", name="pyt")
                for t in range(3):
                    nc.tensor.matmul(p_yt, W_mask[t], hcs[t][:, c0:c1],
                                     start=(t == 0), stop=False)
                nc.tensor.matmul(p_yt, W_dp[bi], xi[g][:, c0:c1],
                                 start=False, stop=True)
                nc.vector.tensor_mul(y[g][rr, c0:c1], p_yt,
                                     siluz[g][rr, c0:c1])
            e_mag(b)               # interleave one DFT M-tile per batch

        e_mag(8)
        # CNN: 3 shifted block-diag matmuls
        for c0, c1 in CH:
            p_cn = pmm.tile([128, NCH], F32, tag="pmm", name="pmm")
            for k in range(3):
                mm512(p_cn, W_cnn[k], xfT, c0 + k, c1 + k,
                      start=(k == 0), stop=(k == 2))
            act(xcnn[:, c0:c1], p_cn, AF.Identity, bias=Vec[V_BCNN])

        # ================= phase D: out_proj -> LN1 -> FFN -> LN2 ==========
        hhat = big.tile([128, N], BF16, tag="hhat", name="hhat")
        h_aff = big.tile([128, N], BF16, tag="h_aff", name="h_aff")
        for c0, c1 in CH:
            p_m = pmm.tile([128, NCH], F32, tag="pmm", name="pmm")
            for g in range(2):
                mm512(p_m[64 * g:64 * g + 64, :], W_op, y[g], c0, c1)
            cent = ws.tile([128, NCH], BF16, tag="cent", name="cent")
            sq = ws.tile([128, NCH], BF16, tag="sq", name="sq")
            act(cent, p_m, AF.Copy)
            act(sq, p_m, AF.Square)
            p_v = pmm.tile([8, NCH], F32, tag="pmm", name="pmm")
            mm512(p_v, W_ones12, sq, 0, NCH)
            sd = ws.tile([8, NCH], BF16, tag="sd", name="sd")
            act(sd, p_v, AF.Ln, bias=Beps)
            inv = ws.tile([8, NCH], BF16, tag="sd", name="inv")
            act(inv, sd, AF.Exp, scale=-0.5)
            p_b = pmm.tile([128, NCH], F32, tag="pmm", name="pmm")
            mm512(p_b, W_bc8, inv, 0, NCH)
            nc.vector.tensor_mul(hhat[:, c0:c1], cent, p_b)
            nc.vector.tensor_scalar(h_aff[:, c0:c1], hhat[:, c0:c1],
                                    Vec[V_G1], Vec[V_B1], OP.mult, OP.add)
        # FFN: gelu via the exact-gelu ACT table (all gelu calls batched)
        s_t = big.tile([128, N], BF16, tag="s_t", name="s_t")
        for q in range(4):
            for c0, c1 in CH:
                p_f = pmm.tile([128, NCH], F32, tag="pmm", name="pmm")
                mm512(p_f, W_ffn1[q], hhat, c0, c1)
                ff_c = ws.tile([128, NCH], BF16, tag="ffch", name="ff_c")
                act(ff_c, p_f, AF.Gelu, bias=Vec[V_BFFN1])
                p_2 = pmm.tile([32, NCH], F32, tag="pmm", name="pmm")
                mm512(p_2, W_ffn2[q], ff_c, 0, NCH)
                rq = slice(32 * q, 32 * q + 32)
                nc.vector.scalar_tensor_tensor(
                    s_t[rq, c0:c1], p_2, Vec[V_BFFN2][rq, :],
                    h_aff[rq, c0:c1], OP.add, OP.add)
        # LN2
        xm_hat = big.tile([128, N], BF16, tag="xm_hat", name="xm_hat")
        for c0, c1 in CH:
            p_c = pmm.tile([128, NCH], F32, tag="pmm", name="pmm")
            mm512(p_c, W_pc, s_t, c0, c1)
            c2 = ws.tile([128, NCH], BF16, tag="cent", name="c2")
            act(c2, p_c, AF.Copy)
            sq2 = ws.tile([128, NCH], BF16, tag="sq", name="sq2")
            act(sq2, p_c, AF.Square)
            p_v2 = pmm.tile([8, NCH], F32, tag="pmm", name="pmm")
            mm512(p_v2, W_ones12, sq2, 0, NCH)
            sd2 = ws.tile([8, NCH], BF16, tag="sd", name="sd2")
            act(sd2, p_v2, AF.Ln, bias=Beps)
            inv2 = ws.tile([8, NCH], BF16, tag="sd", name="inv2")
            act(inv2, sd2, AF.Exp, scale=-0.5)
            p_b2 = pmm.tile([128, NCH], F32, tag="pmm", name="pmm")
            mm512(p_b2, W_bc8, inv2, 0, NCH)
            nc.vector.tensor_mul(xm_hat[:, c0:c1], c2, p_b2)

        # ================= phase F: fusion head =============================
        racc = [sg.tile([4, 1], F32, tag=f"racc{g}", name=f"racc{g}")
                for g in range(2)]
        for g in range(2):
            nc.vector.memset(racc[g], 0.0)
        for c0, c1 in CH:
            p_1 = pmm.tile([128, NCH], F32, tag="pmm", name="pmm")
            mm512(p_1, W_lin1a, xm_hat, c0, c1, start=True, stop=False)
            mm512(p_1, W_lin1b, xcnn, c0, c1, start=False, stop=True)
            mneg = ws.tile([128, NCH], BF16, tag="mneg", name="mneg")
            nc.vector.tensor_scalar(mneg, p_1, Vec[V_BHEAD1], 0.0,
                                    OP.add, OP.min)
            e_t = ws.tile([128, NCH], BF16, tag="e_t", name="e_t")
            act(e_t, mneg, AF.Exp)
            r_t = ws.tile([128, NCH], BF16, tag="mneg", name="r_t")
            act(r_t, p_1, AF.Relu, bias=Vec[V_BHEAD1])
            v_t = ws.tile([128, NCH], BF16, tag="e_t", name="v_t")
            nc.vector.tensor_add(v_t, r_t, e_t)
            for g in range(2):
                p_o2 = pmm.tile([128, NCH], F32, tag="pmm", name="pmm")
                mm512(p_o2, W_lin2[g], v_t, 0, NCH)
                o2c = ws.tile([128, NCH], BF16, tag="mneg", name="o2c")
                act(o2c, p_o2, AF.Identity, bias=Vec[V_BLIN2])
                p_o3 = pmm.tile([4, NCH], F32, tag="pmm", name="pmm")
                mm512(p_o3, W_lin3[g], o2c, 0, NCH)
                o3c = ws.tile([4, NCH], BF16, tag="o3c", name="o3c")
                act(o3c, p_o3, AF.Copy)
                rc = ws.tile([4, 1], F32, tag="rc", name="rc")
                nc.vector.tensor_reduce(rc, o3c, AX.X, OP.add)
                nc.vector.tensor_add(racc[g], racc[g], rc)
        if DBG:
            for k, tl in (("y0", y[0]), ("y1", y[1]), ("hhat", hhat),
                          ("s_t", s_t), ("xm_hat", xm_hat), ("xcnn", xcnn)):
                tf = ws.tile([128, N], BF16, tag="dbgtmp", name=f"dbg{k}",
                             bufs=1)
                nc.vector.tensor_copy(tf, tl)
                nc.sync.dma_start(out=dbg_d[k][:, :], in_=tf)
        for g in range(2):
            res = sg.tile([4, 1], F32, tag=f"res{g}", name=f"res{g}")
            act(res, racc[g], AF.Sigmoid, bias=Bout[0:4, :], scale=1.0 / N)
            nc.sync.dma_start(out=out_d[4 * g:4 * g + 4, :], in_=res)

    # Prefer the combined ln+exp ACT table: hide Exp/Ln from all other
    # tables so the table-load pass lands on natural_log_exp_and_others
    # (availability-only metadata; claiming less than reality is safe).
    import concourse.bacc as bacc_mod
    from concourse import mybir as _mb
    _orig_gat = bacc_mod.get_activation_tables

    def _gat(arch):
        t = {k: set(v) for k, v in _orig_gat(arch).items()}
        for name, s in t.items():
            if name != "natural_log_exp_and_others":
                s.discard(_mb.ActivationFunctionType.Exp)
                s.discard(_mb.ActivationFunctionType.Ln)
        return t

    bacc_mod.get_activation_tables = _gat
    try:
        nc.compile()
    finally:
        bacc_mod.get_activation_tables = _orig_gat
    return nc


# ---------------------------------------------------------------- host side
def _host_prep(inputs):
    f32, f16 = np.float32, np.float16
    x = inputs["x"].astype(f32)
    in_proj_w = inputs["in_proj_w"].astype(f32)
    conv_w = inputs["conv_w"].astype(f32)
    conv_b = inputs["conv_b"].astype(f32)
    x_proj_w = inputs["x_proj_w"].astype(f32)
    dt_w = inputs["dt_w"].astype(f32)
    dt_b = inputs["dt_b"].astype(f32)
    A_log = inputs["A_log"].astype(f32)
    Dp = inputs["Dp"].astype(f32)
    out_proj_w = inputs["out_proj_w"].astype(f32)
    ln1_g, ln1_b = inputs["ln1_g"].astype(f32), inputs["ln1_b"].astype(f32)
    ffn_w1, ffn_b1 = inputs["ffn_w1"].astype(f32), inputs["ffn_b1"].astype(f32)
    ffn_w2, ffn_b2 = inputs["ffn_w2"].astype(f32), inputs["ffn_b2"].astype(f32)
    ffn_ln_g = inputs["ffn_ln_g"].astype(f32)
    ffn_ln_b = inputs["ffn_ln_b"].astype(f32)
    cnn_w, cnn_b = inputs["cnn_w"].astype(f32), inputs["cnn_b"].astype(f32)
    lin1_w, lin1_b = inputs["lin1_w"].astype(f32), inputs["lin1_b"].astype(f32)
    lin2_w, lin2_b = inputs["lin2_w"].astype(f32), inputs["lin2_b"].astype(f32)
    lin3_w, lin3_b = inputs["lin3_w"].astype(f32), inputs["lin3_b"].astype(f32)

    sh = {}
    # fused in_proj + conv:  Wxc[k*12+m, d] = conv_w[d,0,k]*in_proj_w[d,m]
    Wxc = np.einsum('dk,dm->kmd', conv_w[:, 0, :], in_proj_w[:DI]).reshape(48, DI)
    sh["w_xc"] = np.zeros((96, 64), f32)
    sh["w_z"] = np.zeros((96, 64), f32)
    for b2 in range(2):
        sh["w_xc"][48 * b2:48 * b2 + 48, 32 * b2:32 * b2 + 24] = Wxc
        for m in range(DM):
            sh["w_z"][48 * b2 + 36 + m, 32 * b2:32 * b2 + 24] = in_proj_w[DI:, m]
    # x_proj (delta rank-1 folded)
    Wdelta = np.einsum('d,j->jd', dt_w[:, 0], x_proj_w[0])     # [24,24]
    WBC = x_proj_w[1:].T                                       # [24,32]
    sh["w_delta"] = np.zeros((128, 128), f32)
    sh["w_bc"] = np.zeros((128, 128), f32)
    for bi in range(4):
        r = slice(32 * bi, 32 * bi + 24)
        sh["w_delta"][r, 32 * bi:32 * bi + 24] = Wdelta
        sh["w_bc"][r, 32 * bi:32 * bi + 32] = WBC
    # delta broadcast selection: out row 16*dl+n <- delta row 32*bi+8*t+dl
    sh["w_sel"] = np.zeros((4, 3, 128, 128), f32)
    for bi in range(4):
        for t in range(3):
            for dl in range(8):
                for n in range(DS):
                    sh["w_sel"][bi, t, 32 * bi + 8 * t + dl, 16 * dl + n] = 1.0
    # B/C broadcast selection: out row 16*dl+n <- BC row 32*bi+16*u+n
    sh["w_selbc"] = np.zeros((4, 2, 128, 128), f32)
    for bi in range(4):
        for u in range(2):
            for dl in range(8):
                for n in range(DS):
                    sh["w_selbc"][bi, u, 32 * bi + 16 * u + n,
                                  16 * dl + n] = 1.0
    # out_proj with centering fold
    Pc = np.eye(DM, dtype=f32) - f32(1.0 / DM)
    WopT = (Pc @ out_proj_w).T                                 # [24,12]
    sh["w_op"] = np.zeros((128, 64), f32)
    for bi in range(4):
        sh["w_op"][32 * bi:32 * bi + 24, 16 * bi:16 * bi + 12] = WopT
    sh["w_ones12"] = np.zeros((128, 8), f32)
    sh["w_bc8"] = np.zeros((8, 128), f32)
    for b in range(8):
        sh["w_ones12"][16 * b:16 * b + 12, b] = f32(1.0 / DM)
        sh["w_bc8"][b, 16 * b:16 * b + 16] = 1.0
    # ffn
    W1p = (ffn_w1 * ln1_g[None, :]).T                          # [12,48]
    b1p = ffn_b1 + ffn_w1 @ ln1_b
    sh["w_ffn1"] = np.zeros((4, 128, 128), f32)
    sh["w_ffn2"] = np.zeros((4, 128, 32), f32)
    for q in range(4):
        for b2 in range(2):
            b = 2 * q + b2
            sh["w_ffn1"][q, 16 * b:16 * b + 12, 64 * b2:64 * b2 + 48] = W1p
            sh["w_ffn2"][q, 64 * b2:64 * b2 + 48,
                         16 * b2:16 * b2 + 12] = ffn_w2.T
    sh["w_pc"] = np.zeros((128, 128), f32)
    W1aT = (lin1_w[:, :DM] * ffn_ln_g[None, :]).T              # [12,12]
    W1bT = lin1_w[:, DM:].T
    sh["w_lin1a"] = np.zeros((128, 128), f32)
    sh["w_lin1b"] = np.zeros((128, 128), f32)
    for b in range(8):
        r = slice(16 * b, 16 * b + 12)
        sh["w_pc"][r, r] = Pc
        sh["w_lin1a"][r, r] = W1aT
        sh["w_lin1b"][r, r] = W1bT
    b1h = lin1_b + lin1_w[:, :DM] @ ffn_ln_b
    b2p = lin2_b - lin2_w.sum(axis=1)
    sh["w_lin2"] = np.zeros((2, 128, 128), f32)
    sh["w_lin3"] = np.zeros((2, 128, 4), f32)
    for g in range(2):
        for bi in range(4):
            b = 4 * g + bi
            sh["w_lin2"][g, 16 * b:16 * b + 12,
                         32 * bi:32 * bi + 20] = lin2_w.T
            sh["w_lin3"][g, 32 * bi:32 * bi + 20, bi] = lin3_w[0]
    sh["w_cnn"] = np.zeros((3, 96, 128), f16)
    for k in range(3):
        for b in range(8):
            sh["w_cnn"][k, 12 * b:12 * b + 12,
                        16 * b:16 * b + 12] = cnn_w[:, :, k].T.astype(f16)
    # scan masks and A scales
    sh["w_mask"] = np.zeros((3, 128, 32), np.float32)
    sh["sc_negA"] = np.zeros((3, 128, 1), f32)
    Asc = -np.exp(A_log)                                       # [24,16]
    for t in range(3):
        for dl in range(8):
            for n in range(DS):
                sh["w_mask"][t, 16 * dl + n, 8 * t + dl] = 1.0
                sh["sc_negA"][t, 16 * dl + n, 0] = Asc[8 * t + dl, n]
    sh["ident"] = np.eye(128, dtype=f16)
    sh["identj"] = np.eye(128, dtype=f16)[::-1].copy()

    def pack(v, blk, nblk):
        o = np.zeros(128, f32)
        for i in range(nblk):
            o[blk * i:blk * i + len(v)] = v
        return o

    vecs = np.zeros((128, 11), f32)
    bconv64 = np.zeros(64, f32)
    bconv64[0:24] = conv_b
    bconv64[32:56] = conv_b
    vecs[:, 0] = np.concatenate([bconv64, bconv64])
    vecs[:, 1] = pack(dt_b, 32, 4)
    vecs[:, 2] = pack(Dp, 32, 4)
    vecs[:, 3] = pack(ln1_g, 16, 8)
    vecs[:, 4] = pack(ln1_b, 16, 8)
    vecs[:, 5] = pack(b1p, 64, 2)
    vecs[:, 6] = pack(ffn_b2, 16, 8)
    vecs[:, 7] = pack(b1h, 16, 8)
    vecs[:, 8] = pack(b2p, 32, 4)
    vecs[:, 9] = pack(cnn_b, 16, 8)
    vecs[:, 10] = pack(b1p, 64, 2)
    sh["vecs"] = vecs
    sh["w_dp"] = np.zeros((4, 128, 32), f32)
    for bi in range(4):
        for c in range(DI):
            sh["w_dp"][bi, 32 * bi + c, c] = Dp[c]
    sh["b_out"] = np.full((8, 1), lin3_b[0], f32)
    sh["b_eps"] = np.full((8, 1), 1e-12, f32)
    # DFT matrices, tiled [mt, cs, kt, 128, 128]
    t_ = np.arange(L, dtype=np.float64)
    f_ = np.arange(NF, dtype=np.float64)
    ang = (2 * np.pi / L) * np.outer(f_, t_)
    wc = np.cos(ang)
    wsn = np.sin(ang)
    wc[1025:] = 0.0
    wsn[1025:] = 0.0
    wdft = np.zeros((NMT, 2, 128, NKT * 128), f16)
    for mt in range(NMT):
        for kt in range(NKT):
            blkc = wc[128 * mt:128 * mt + 128, 128 * kt:128 * kt + 128]
            blks = wsn[128 * mt:128 * mt + 128, 128 * kt:128 * kt + 128]
            wdft[mt, 0, :, 128 * kt:128 * kt + 128] = blkc.T.astype(f16)
            wdft[mt, 1, :, 128 * kt:128 * kt + 128] = blks.T.astype(f16)
    sh["wdft"] = wdft

    # per-core data
    per_core = []
    for c in range(NCORES):
        xl = x[BL * c:BL * c + BL]                             # [8,2048,12]
        xs = np.zeros((4, 96, N), f32)
        for j in range(4):
            for b2 in range(2):
                xb = xl[2 * j + b2]                            # [2048,12]
                for k in range(4):
                    shf = 3 - k
                    r0 = 48 * b2 + 12 * k
                    if shf == 0:
                        xs[j, r0:r0 + 12, :] = xb.T
                    else:
                        xs[j, r0:r0 + 12, shf:] = xb[:-shf].T
        xt = np.zeros((128, NKT * 96), f16)
        for kt in range(NKT):
            xt[:, 96 * kt:96 * kt + 96] = \
                xl[:, 128 * kt:128 * kt + 128].transpose(1, 0, 2) \
                .reshape(128, 96).astype(f16)
        import ml_dtypes as _md
        per_core.append({"xs": xs.astype(_md.bfloat16), "xt": xt})
    return sh, per_core


def kernel(**inputs):
    import ml_dtypes
    sh, per_core = _host_prep(inputs)
    if "nc" not in _CACHE:
        _CACHE["nc"] = _build_module()
    nc = _CACHE["nc"]
    sh = dict(sh)
    for k in ("w_mask", "w_xc", "w_z", "w_delta", "w_bc", "w_dp", "w_sel",
          "w_selbc",
          "w_op", "w_ones12", "w_bc8", "w_ffn1", "w_ffn2", "w_pc",
          "w_lin1a", "w_lin1b", "w_lin2", "w_lin3"):
        sh[k] = sh[k].astype(ml_dtypes.bfloat16)
    in_maps = [{**sh, **pc} for pc in per_core]
    from concourse.bass_utils import run_bass_kernel_spmd
    res = run_bass_kernel_spmd(nc, in_maps, core_ids=list(range(NCORES)))
    outs = [res.results[c]["out"].reshape(BL) for c in range(NCORES)]
    return np.concatenate(outs).astype(np.float32)


# revision 15
# speedup vs baseline: 2.2653x; 1.0971x over previous
"""Trainium2 Bass kernel for nn_Net_90331752170289 (Mamba block + FFT/CNN + fusion head).

Strategy: pure data parallelism over batch (8 batches per core on 8 cores).
Per-core layout: partitions carry (batch, channel) blocks padded to the
32-partition grid, free dim = time (2048 per batch).  The Mamba selective
scan runs as hardware tensor_tensor_scan instructions over (d,n)-partition
tiles in bf16; the FFT is a DFT matmul in fp16 against host-built cos/sin
matrices (half spectrum + mirror via reversed-identity PE transposes); all
small linear layers are block-diagonal float32r matmuls with LN/conv/affine
folds precomputed on the host.

v2: the delta broadcast for the scan is generated on the tensor engine
(selection matmul into PSUM, consumed directly by the ACT exp) instead of
DMA replication — the v1 DMA-broadcast scheme moved ~31MB SBUF->SBUF and
saturated all 8 DMA engines for ~80% of the kernel.  dx/B/C broadcasts stay
on DMA (~20MB).  hc/dBx are single full-tile vector ops, phase A/D psum
tiles are packed to 128 partitions, and the FFN gelu uses the exact-gelu
ACT table directly (table-set-batched phase D) instead of an erf composition.

Block layouts (per core, 8 local batches b, groups g=b//4, bi=b%4):
  X24 tensors (xi, siluz, delta, dx, y): [128, 2048] per g, row = 32*bi + ch
  BC: [128, 2048] per g, row = 32*bi + r (r<16 B, else C)
  X12 tensors (hhat, h_aff, s_t, xm_hat, xcnn): [128, 2048], row = 16*b + m
  scan tiles: [128, 2048] per (b, dn-tile), row = 16*dl + n, d = 8*tile + dl
"""
import numpy as np

B, L, DM = 64, 2048, 12
DI, DS, DC = 24, 16, 4
NCORES = 8
BL = B // NCORES          # 8 local batches per core
N = L                     # free dim per batch
NCH = 1024                # psum chunk (2 banks)
NCY = 512                 # scan-output psum chunk
NF = 1152                 # padded rfft bins (valid 0..1024)
NMT = NF // 128           # 9 DFT M-tiles
NKT = L // 128            # 16 DFT K-tiles

_CACHE = {}

# weight-pack manifests: (name, partitions, cols)
MAN_BF = ([("w_xc", 96, 64), ("w_z", 96, 64), ("w_delta", 128, 128),
           ("w_bc", 128, 128)]
          + [(f"w_sel_{bi}_{t}", 128, 128) for bi in range(4)
             for t in range(3)]
          + [(f"w_selbc_{bi}_{u}", 128, 128) for bi in range(4)
             for u in range(2)]
          + [("w_op", 128, 64), ("w_ones12", 128, 8), ("w_bc8", 8, 128)]
          + [(f"w_ffn1_{q}", 128, 128) for q in range(4)]
          + [(f"w_ffn2_{q}", 128, 32) for q in range(4)]
          + [("w_pc", 128, 128), ("w_lin1a", 128, 128),
             ("w_lin1b", 128, 128)]
          + [(f"w_lin2_{g}", 128, 128) for g in range(2)]
          + [(f"w_lin3_{g}", 128, 4) for g in range(2)]
          + [(f"w_dp_{bi}", 128, 32) for bi in range(4)]
          + [(f"w_mask_{t}", 128, 32) for t in range(3)])
MAN_F16 = ([(f"w_cnn_{k}", 96, 128) for k in range(3)]
           + [("ident", 128, 128), ("identj", 128, 128)])
MAN_F32 = ([(f"sc_negA_{t}", 128, 1) for t in range(3)]
           + [("vecs", 128, 11), ("b_out", 8, 1), ("b_eps", 8, 1)])
NBF = sum(c for _, _, c in MAN_BF)
NF16 = sum(c for _, _, c in MAN_F16)
NF32 = sum(c for _, _, c in MAN_F32)


# ---------------------------------------------------------------- device code
def _build_module():
    import concourse.bacc as bacc
    import concourse.bass as bass
    import concourse.tile as tile
    from concourse import mybir
    from contextlib import ExitStack

    F32 = mybir.dt.float32
    F32R = mybir.dt.float32r
    F16 = mybir.dt.float16
    BF16 = mybir.dt.bfloat16
    AF = mybir.ActivationFunctionType
    OP = mybir.AluOpType
    AX = mybir.AxisListType

    nc = bacc.Bacc("TRN2", target_bir_lowering=False, debug=False)

    def din(name, shape, dt=F32R):
        return nc.dram_tensor(name, shape, dt, kind="ExternalInput")

    # per-core data
    xs_d = din("xs", [4, 96, N], BF16)                  # in_proj rhs, per b-pair
    xt_d = din("xt", [128, NKT * 96], F16)        # DFT rhs, kt-major cols
    wdft_d = din("wdft", [NMT, 2, 128, NKT * 128], F16)
    # folded weights, packed per dtype (one DMA each; loading ~35 small
    # tensors individually cost ~40us of serial small-descriptor DMA)
    wpk_bf_d = din("wpk_bf", [128, NBF], BF16)
    wpk_f16_d = din("wpk_f16", [128, NF16], F16)
    wpk_f32_d = din("wpk_f32", [128, NF32], F32)
    (V_BCONV, V_BDT, V_SDP, V_G1, V_B1, V_BFFN1, V_BFFN2, V_BHEAD1,
     V_BLIN2, V_BCNN, V_BFFN1S) = range(11)

    out_d = nc.dram_tensor("out", [8, 1], F32, kind="ExternalOutput")
    DBG = _CACHE.get("dbg", False)
    if DBG:
        dbg_d = {k: nc.dram_tensor(f"dbg_{k}", [128, N], BF16,
                                   kind="ExternalOutput")
                 for k in ("y0", "y1", "hhat", "s_t", "xm_hat", "xcnn")}

    with tile.TileContext(nc) as tc, ExitStack() as ctx:
        sg = ctx.enter_context(tc.tile_pool(name="singles", bufs=1))
        ws = ctx.enter_context(tc.tile_pool(name="work", bufs=2))
        big = ctx.enter_context(tc.tile_pool(name="big", bufs=1))
        pmm = ctx.enter_context(tc.tile_pool(name="pmm", bufs=2, space="PSUM"))
        py = ctx.enter_context(tc.tile_pool(name="py", bufs=2, space="PSUM"))
        pdft = ctx.enter_context(tc.tile_pool(name="pdft", bufs=2, space="PSUM"))

        def load(dram_ap, shape, dt, tag, pool=sg):
            t = pool.tile(shape, dt, tag=tag, name=tag)
            nc.sync.dma_start(out=t, in_=dram_ap)
            return t

        def act(out, in_, func, **kw):
            return nc.scalar.activation(out, in_, func, **kw)

        def mm512(p, lhsT, rhs, c0, c1, start=True, stop=True):
            # matmul into a [*, c1-c0] psum tile in 512-wide pieces
            for o in range(0, c1 - c0, 512):
                w = min(512, c1 - c0 - o)
                nc.tensor.matmul(p[:, o:o + w], lhsT, rhs[:, c0 + o:c0 + o + w],
                                 start=start, stop=stop)

        # ---- load weights/constants into SBUF (3 packed DMAs)
        PkBF = sg.tile([128, NBF], BF16, tag="wpk_bf", name="wpk_bf")
        nc.sync.dma_start(out=PkBF, in_=wpk_bf_d[:, :])
        PkF16 = sg.tile([128, NF16], F16, tag="wpk_f16", name="wpk_f16")
        nc.sync.dma_start(out=PkF16, in_=wpk_f16_d[:, :])
        PkF32 = sg.tile([128, NF32], F32, tag="wpk_f32", name="wpk_f32")
        nc.sync.dma_start(out=PkF32, in_=wpk_f32_d[:, :])

        def mkslices(pk, manifest):
            out, o = {}, 0
            for name, p, c in manifest:
                out[name] = pk[0:p, o:o + c]
                o += c
            return out

        SB = mkslices(PkBF, MAN_BF)
        S16 = mkslices(PkF16, MAN_F16)
        S32 = mkslices(PkF32, MAN_F32)
        W_xc, W_z = SB["w_xc"], SB["w_z"]
        W_delta, W_bc = SB["w_delta"], SB["w_bc"]
        W_sel = [[SB[f"w_sel_{bi}_{t}"] for t in range(3)] for bi in range(4)]
        W_selBC = [[SB[f"w_selbc_{bi}_{u}"] for u in range(2)]
                   for bi in range(4)]
        W_op, W_ones12, W_bc8 = SB["w_op"], SB["w_ones12"], SB["w_bc8"]
        W_ffn1 = [SB[f"w_ffn1_{q}"] for q in range(4)]
        W_ffn2 = [SB[f"w_ffn2_{q}"] for q in range(4)]
        W_pc = SB["w_pc"]
        W_lin1a, W_lin1b = SB["w_lin1a"], SB["w_lin1b"]
        W_lin2 = [SB[f"w_lin2_{g}"] for g in range(2)]
        W_lin3 = [SB[f"w_lin3_{g}"] for g in range(2)]
        W_dp = [SB[f"w_dp_{bi}"] for bi in range(4)]
        W_mask = [SB[f"w_mask_{t}"] for t in range(3)]
        W_cnn = [S16[f"w_cnn_{k}"] for k in range(3)]
        Ident, IdentJ = S16["ident"], S16["identj"]
        ScA = [S32[f"sc_negA_{t}"] for t in range(3)]
        Vec_t = S32["vecs"]
        Vec = [Vec_t[:, i:i + 1] for i in range(11)]
        Bout, Beps = S32["b_out"], S32["b_eps"]

        # ---- persistent activations
        xi = [big.tile([128, N], BF16, tag=f"xi{g}", name=f"xi{g}")
              for g in range(2)]
        siluz = [big.tile([128, N], BF16, tag=f"siluz{g}", name=f"siluz{g}")
                 for g in range(2)]
        delta = [big.tile([128, N], BF16, tag=f"delta{g}", name=f"delta{g}")
                 for g in range(2)]
        dx = [big.tile([128, N], BF16, tag=f"dx{g}", name=f"dx{g}")
              for g in range(2)]
        BC = [big.tile([128, N], BF16, tag=f"bc{g}", name=f"bc{g}")
              for g in range(2)]
        y = [big.tile([128, N], BF16, tag=f"y{g}", name=f"y{g}")
             for g in range(2)]
        xcnn = big.tile([128, N], BF16, tag="xcnn", name="xcnn")
        xfT = big.tile([96, N + 2], F16, tag="xfT", name="xfT")

        CH = [(c * NCH, (c + 1) * NCH) for c in range(N // NCH)]

        # ================= phase A: fused in_proj + causal conv, silu =======
        for g in range(2):
            xsp = [ws.tile([96, N], BF16, tag="pairA", name="xsp")
                   for j in range(2)]
            for j in range(2):
                nc.sync.dma_start(out=xsp[j], in_=xs_d[2 * g + j])
            for c0, c1 in CH:
                p_xc = pmm.tile([128, NCH], F32, tag="pmm", name="pmm")
                p_z = pmm.tile([128, NCH], F32, tag="pmm", name="pmm")
                for j in range(2):
                    jj = slice(64 * j, 64 * j + 64)
                    mm512(p_xc[jj, :], W_xc, xsp[j], c0, c1)
                    mm512(p_z[jj, :], W_z, xsp[j], c0, c1)
                act(xi[g][:, c0:c1], p_xc, AF.Silu, bias=Vec[V_BCONV])
                act(siluz[g][:, c0:c1], p_z, AF.Silu)

        # ================= phase B: x_proj (delta folded), dx ==============
        for g in range(2):
            for c0, c1 in CH:
                p_d = pmm.tile([128, NCH], F32, tag="pmm", name="pmm")
                mm512(p_d, W_delta, xi[g], c0, c1)
                edt = ws.tile([128, NCH], F32, tag="sgm", name="edt")
                act(edt, p_d, AF.Exp, bias=Vec[V_BDT])
                act(delta[g][:, c0:c1], edt, AF.Ln, bias=1.0)
                p_bc = pmm.tile([128, NCH], F32, tag="pmm", name="pmm")
                mm512(p_bc, W_bc, xi[g], c0, c1)
                act(BC[g][:, c0:c1], p_bc, AF.Copy)
            nc.vector.tensor_mul(dx[g], delta[g], xi[g])

        # ================= phase E setup (DFT |FFT|, interleaved into C) ====
        xt_sb = sg.tile([128, NKT * 96], F16, tag="xt", name="xt")
        nc.sync.dma_start(out=xt_sb, in_=xt_d[:, :])
        xf = [sg.tile([128, 96], F16, tag=f"xf{m}", name=f"xf{m}")
              for m in range(NMT)]
        nc.vector.memset(xfT[:, 0:1], 0.0)

        def e_mag(mt):
            # one DFT M-tile: |FFT| magnitudes + transpose/mirror into xfT
            mags = []
            for cs in range(2):
                p_acc = pdft.tile([128, 96], F32, tag="pdft", name="pdft")
                wsl = ws.tile([128, NKT * 128], F16, tag="wsl", name="wsl")
                for hh in range(2):
                    nc.sync.dma_start(
                        out=wsl[:, 1024 * hh:1024 * hh + 1024],
                        in_=wdft_d[mt, cs, :, 1024 * hh:1024 * hh + 1024])
                for kt in range(NKT):
                    nc.tensor.matmul(p_acc, wsl[:, 128 * kt:128 * kt + 128],
                                     xt_sb[:, 96 * kt:96 * kt + 96],
                                     start=(kt == 0), stop=(kt == NKT - 1))
                m_sq = ws.tile([128, 96], F32, tag=f"m_sq{cs}", name="m_sq")
                act(m_sq, p_acc, AF.Square)
                mags.append(m_sq)
            nc.vector.scalar_tensor_tensor(mags[0], mags[0], 1e-20,
                                           mags[1], OP.add, OP.add)
            lnm = ws.tile([128, 96], F32, tag="m_sq1", name="lnm")
            act(lnm, mags[0], AF.Ln)
            act(xf[mt], lnm, AF.Exp, scale=0.5)
            p_t = pdft.tile([96, 128], F16, tag="pdft", name="pdft")
            nc.tensor.transpose(p_t, xf[mt], Ident)
            if mt < 8:
                act(xfT[:, 1 + 128 * mt:1 + 128 * mt + 128], p_t, AF.Copy)
            else:
                # only f=1024 is a valid forward bin; cols 1026.. belong to
                # the mirrored half written by mt<=7
                act(xfT[:, 1025:1026], p_t[:, 0:1], AF.Copy)
            if mt < 8:             # mirrored half: t = 2048 - f, f=128*mt+j
                p_r = pdft.tile([96, 128], F16, tag="pdft", name="pdft")
                nc.tensor.transpose(p_r, xf[mt], IdentJ)
                act(xfT[:, 1922 - 128 * mt:1922 - 128 * mt + 128],
                    p_r, AF.Copy)
                if mt == 0:
                    # mirror of mt=0 writes through col N+1; re-zero the
                    # conv right-pad column after it
                    nc.vector.memset(xfT[:, N + 1:N + 2], 0.0)

        # ================= phase C: selective scan ==========================
        for b in range(BL):
            g, bi = b // 4, b % 4
            # B/C broadcast: selection matmul on the PE + ACT copy out of
            # PSUM (the v1/v2 DMA doubling chains serialized on the SW DMA
            # queues and stalled the scan ~19us per batch pair).
            Bbc = ws.tile([128, N], BF16, tag="Bbc", name="Bbc")
            Cbc = ws.tile([128, N], BF16, tag="Cbc", name="Cbc")
            for u, dst in ((0, Bbc), (1, Cbc)):
                for c0, c1 in CH:
                    p_bb = pmm.tile([128, NCH], F32, tag="pmm", name="pmm")
                    mm512(p_bb, W_selBC[bi][u], BC[g], c0, c1)
                    act(dst[:, c0:c1], p_bb, AF.Copy)
            hcs = []
            for t in range(3):
                r0 = 32 * bi + 8 * t
                # delta broadcast on the PE: selection matmul into PSUM,
                # then exp(A*delta) straight out of PSUM into SBUF bf16.
                a_t = ws.tile([128, N], BF16, tag="a_t", name="a_t")
                for c0, c1 in CH:
                    p_db = pmm.tile([128, NCH], F32, tag="pmm", name="pmm")
                    mm512(p_db, W_sel[bi][t], delta[g], c0, c1)
                    act(a_t[:, c0:c1], p_db, AF.Exp, scale=ScA[t])
                # dx broadcast via one replication DMA
                dx_sl = dx[g][r0:r0 + 8, :]
                dxbc = ws.tile([128, N], BF16, tag="dxbc", name="dxbc")
                nc.sync.dma_start(
                    out=dxbc,
                    in_=bass.AP(tensor=dx_sl.tensor, offset=dx_sl.offset,
                                ap=[dx_sl.ap[0], [0, 16], dx_sl.ap[1]]))
                dBx = ws.tile([128, N], BF16, tag="dBx", name="dBx")
                nc.vector.tensor_mul(dBx, dxbc, Bbc)
                h_t = ws.tile([128, N], BF16, tag="h_t", name="h_t")
                nc.vector.tensor_tensor_scan(h_t, a_t, dBx, 0.0,
                                             OP.mult, OP.add)
                hc = ws.tile([128, N], BF16, tag=f"hc{t}", name="hc", bufs=1)
                nc.vector.tensor_mul(hc, h_t, Cbc)
                hcs.append(hc)
            # y = (ys + xi*Dp) * silu(z)
            rr = slice(32 * bi, 32 * bi + 32)
            for c0 in range(0, N, NCY):
                c1 = c0 + NCY
                p_yt = py.tile([32, NCY], F32, tag="pyt", name="pyt")
                for t in range(3):
                    nc.tensor.matmul(p_yt, W_mask[t], hcs[t][:, c0:c1],
                                     start=(t == 0), stop=False)
                nc.tensor.matmul(p_yt, W_dp[bi], xi[g][:, c0:c1],
                                 start=False, stop=True)
                nc.vector.tensor_mul(y[g][rr, c0:c1], p_yt,
                                     siluz[g][rr, c0:c1])
            e_mag(b)               # interleave one DFT M-tile per batch

        e_mag(8)
        # CNN: 3 shifted block-diag matmuls
        for c0, c1 in CH:
            p_cn = pmm.tile([128, NCH], F32, tag="pmm", name="pmm")
            for k in range(3):
                mm512(p_cn, W_cnn[k], xfT, c0 + k, c1 + k,
                      start=(k == 0), stop=(k == 2))
            act(xcnn[:, c0:c1], p_cn, AF.Identity, bias=Vec[V_BCNN])

        # ================= phase D: out_proj -> LN1 -> FFN -> LN2 ==========
        hhat = big.tile([128, N], BF16, tag="hhat", name="hhat")
        h_aff = big.tile([128, N], BF16, tag="h_aff", name="h_aff")
        for c0, c1 in CH:
            p_m = pmm.tile([128, NCH], F32, tag="pmm", name="pmm")
            for g in range(2):
                mm512(p_m[64 * g:64 * g + 64, :], W_op, y[g], c0, c1)
            cent = ws.tile([128, NCH], BF16, tag="cent", name="cent")
            sq = ws.tile([128, NCH], BF16, tag="sq", name="sq")
            act(cent, p_m, AF.Copy)
            act(sq, p_m, AF.Square)
            p_v = pmm.tile([8, NCH], F32, tag="pmm", name="pmm")
            mm512(p_v, W_ones12, sq, 0, NCH)
            sd = ws.tile([8, NCH], BF16, tag="sd", name="sd")
            act(sd, p_v, AF.Ln, bias=Beps)
            inv = ws.tile([8, NCH], BF16, tag="sd", name="inv")
            act(inv, sd, AF.Exp, scale=-0.5)
            p_b = pmm.tile([128, NCH], F32, tag="pmm", name="pmm")
            mm512(p_b, W_bc8, inv, 0, NCH)
            nc.vector.tensor_mul(hhat[:, c0:c1], cent, p_b)
            nc.vector.tensor_scalar(h_aff[:, c0:c1], hhat[:, c0:c1],
                                    Vec[V_G1], Vec[V_B1], OP.mult, OP.add)
        # FFN: gelu via the exact-gelu ACT table (all gelu calls batched)
        s_t = big.tile([128, N], BF16, tag="s_t", name="s_t")
        for q in range(4):
            for c0, c1 in CH:
                p_f = pmm.tile([128, NCH], F32, tag="pmm", name="pmm")
                mm512(p_f, W_ffn1[q], hhat, c0, c1)
                ff_c = ws.tile([128, NCH], BF16, tag="ffch", name="ff_c")
                act(ff_c, p_f, AF.Gelu, bias=Vec[V_BFFN1])
                p_2 = pmm.tile([32, NCH], F32, tag="pmm", name="pmm")
                mm512(p_2, W_ffn2[q], ff_c, 0, NCH)
                rq = slice(32 * q, 32 * q + 32)
                nc.vector.scalar_tensor_tensor(
                    s_t[rq, c0:c1], p_2, Vec[V_BFFN2][rq, :],
                    h_aff[rq, c0:c1], OP.add, OP.add)
        # LN2
        xm_hat = big.tile([128, N], BF16, tag="xm_hat", name="xm_hat")
        for c0, c1 in CH:
            p_c = pmm.tile([128, NCH], F32, tag="pmm", name="pmm")
            mm512(p_c, W_pc, s_t, c0, c1)
            c2 = ws.tile([128, NCH], BF16, tag="cent", name="c2")
            act(c2, p_c, AF.Copy)
            sq2 = ws.tile([128, NCH], BF16, tag="sq", name="sq2")
            act(sq2, p_c, AF.Square)
            p_v2 = pmm.tile([8, NCH], F32, tag="pmm", name="pmm")
            mm512(p_v2, W_ones12, sq2, 0, NCH)
            sd2 = ws.tile([8, NCH], BF16, tag="sd", name="sd2")
            act(sd2, p_v2, AF.Ln, bias=Beps)
            inv2 = ws.tile([8, NCH], BF16, tag="sd", name="inv2")
            act(inv2, sd2, AF.Exp, scale=-0.5)
            p_b2 = pmm.tile([128, NCH], F32, tag="pmm", name="pmm")
            mm512(p_b2, W_bc8, inv2, 0, NCH)
            nc.vector.tensor_mul(xm_hat[:, c0:c1], c2, p_b2)

        # ================= phase F: fusion head =============================
        racc = [sg.tile([4, 1], F32, tag=f"racc{g}", name=f"racc{g}")
                for g in range(2)]
        for g in range(2):
            nc.vector.memset(racc[g], 0.0)
        for c0, c1 in CH:
            p_1 = pmm.tile([128, NCH], F32, tag="pmm", name="pmm")
            mm512(p_1, W_lin1a, xm_hat, c0, c1, start=True, stop=False)
            mm512(p_1, W_lin1b, xcnn, c0, c1, start=False, stop=True)
            mneg = ws.tile([128, NCH], BF16, tag="mneg", name="mneg")
            nc.vector.tensor_scalar(mneg, p_1, Vec[V_BHEAD1], 0.0,
                                    OP.add, OP.min)
            e_t = ws.tile([128, NCH], BF16, tag="e_t", name="e_t")
            act(e_t, mneg, AF.Exp)
            r_t = ws.tile([128, NCH], BF16, tag="mneg", name="r_t")
            act(r_t, p_1, AF.Relu, bias=Vec[V_BHEAD1])
            v_t = ws.tile([128, NCH], BF16, tag="e_t", name="v_t")
            nc.vector.tensor_add(v_t, r_t, e_t)
            for g in range(2):
                p_o2 = pmm.tile([128, NCH], F32, tag="pmm", name="pmm")
                mm512(p_o2, W_lin2[g], v_t, 0, NCH)
                o2c = ws.tile([128, NCH], BF16, tag="mneg", name="o2c")
                act(o2c, p_o2, AF.Identity, bias=Vec[V_BLIN2])
                p_o3 = pmm.tile([4, NCH], F32, tag="pmm", name="pmm")
                mm512(p_o3, W_lin3[g], o2c, 0, NCH)
                o3c = ws.tile([4, NCH], BF16, tag="o3c", name="o3c")
                act(o3c, p_o3, AF.Copy)
                rc = ws.tile([4, 1], F32, tag="rc", name="rc")
                nc.vector.tensor_reduce(rc, o3c, AX.X, OP.add)
                nc.vector.tensor_add(racc[g], racc[g], rc)
        if DBG:
            for k, tl in (("y0", y[0]), ("y1", y[1]), ("hhat", hhat),
                          ("s_t", s_t), ("xm_hat", xm_hat), ("xcnn", xcnn)):
                tf = ws.tile([128, N], BF16, tag="dbgtmp", name=f"dbg{k}",
                             bufs=1)
                nc.vector.tensor_copy(tf, tl)
                nc.sync.dma_start(out=dbg_d[k][:, :], in_=tf)
        for g in range(2):
            res = sg.tile([4, 1], F32, tag=f"res{g}", name=f"res{g}")
            act(res, racc[g], AF.Sigmoid, bias=Bout[0:4, :], scale=1.0 / N)
            nc.sync.dma_start(out=out_d[4 * g:4 * g + 4, :], in_=res)

    # Prefer the combined ln+exp ACT table: hide Exp/Ln from all other
    # tables so the table-load pass lands on natural_log_exp_and_others
    # (availability-only metadata; claiming less than reality is safe).
    import concourse.bacc as bacc_mod
    from concourse import mybir as _mb
    _orig_gat = bacc_mod.get_activation_tables

    def _gat(arch):
        t = {k: set(v) for k, v in _orig_gat(arch).items()}
        for name, s in t.items():
            if name != "natural_log_exp_and_others":
                s.discard(_mb.ActivationFunctionType.Exp)
                s.discard(_mb.ActivationFunctionType.Ln)
        return t

    bacc_mod.get_activation_tables = _gat
    try:
        nc.compile()
    finally:
        bacc_mod.get_activation_tables = _orig_gat
    return nc


# ---------------------------------------------------------------- host side
def _host_prep(inputs):
    f32, f16 = np.float32, np.float16
    x = inputs["x"].astype(f32)
    in_proj_w = inputs["in_proj_w"].astype(f32)
    conv_w = inputs["conv_w"].astype(f32)
    conv_b = inputs["conv_b"].astype(f32)
    x_proj_w = inputs["x_proj_w"].astype(f32)
    dt_w = inputs["dt_w"].astype(f32)
    dt_b = inputs["dt_b"].astype(f32)
    A_log = inputs["A_log"].astype(f32)
    Dp = inputs["Dp"].astype(f32)
    out_proj_w = inputs["out_proj_w"].astype(f32)
    ln1_g, ln1_b = inputs["ln1_g"].astype(f32), inputs["ln1_b"].astype(f32)
    ffn_w1, ffn_b1 = inputs["ffn_w1"].astype(f32), inputs["ffn_b1"].astype(f32)
    ffn_w2, ffn_b2 = inputs["ffn_w2"].astype(f32), inputs["ffn_b2"].astype(f32)
    ffn_ln_g = inputs["ffn_ln_g"].astype(f32)
    ffn_ln_b = inputs["ffn_ln_b"].astype(f32)
    cnn_w, cnn_b = inputs["cnn_w"].astype(f32), inputs["cnn_b"].astype(f32)
    lin1_w, lin1_b = inputs["lin1_w"].astype(f32), inputs["lin1_b"].astype(f32)
    lin2_w, lin2_b = inputs["lin2_w"].astype(f32), inputs["lin2_b"].astype(f32)
    lin3_w, lin3_b = inputs["lin3_w"].astype(f32), inputs["lin3_b"].astype(f32)

    sh = {}
    # fused in_proj + conv:  Wxc[k*12+m, d] = conv_w[d,0,k]*in_proj_w[d,m]
    Wxc = np.einsum('dk,dm->kmd', conv_w[:, 0, :], in_proj_w[:DI]).reshape(48, DI)
    sh["w_xc"] = np.zeros((96, 64), f32)
    sh["w_z"] = np.zeros((96, 64), f32)
    for b2 in range(2):
        sh["w_xc"][48 * b2:48 * b2 + 48, 32 * b2:32 * b2 + 24] = Wxc
        for m in range(DM):
            sh["w_z"][48 * b2 + 36 + m, 32 * b2:32 * b2 + 24] = in_proj_w[DI:, m]
    # x_proj (delta rank-1 folded)
    Wdelta = np.einsum('d,j->jd', dt_w[:, 0], x_proj_w[0])     # [24,24]
    WBC = x_proj_w[1:].T                                       # [24,32]
    sh["w_delta"] = np.zeros((128, 128), f32)
    sh["w_bc"] = np.zeros((128, 128), f32)
    for bi in range(4):
        r = slice(32 * bi, 32 * bi + 24)
        sh["w_delta"][r, 32 * bi:32 * bi + 24] = Wdelta
        sh["w_bc"][r, 32 * bi:32 * bi + 32] = WBC
    # delta broadcast selection: out row 16*dl+n <- delta row 32*bi+8*t+dl
    sh["w_sel"] = np.zeros((4, 3, 128, 128), f32)
    for bi in range(4):
        for t in range(3):
            for dl in range(8):
                for n in range(DS):
                    sh["w_sel"][bi, t, 32 * bi + 8 * t + dl, 16 * dl + n] = 1.0
    # B/C broadcast selection: out row 16*dl+n <- BC row 32*bi+16*u+n
    sh["w_selbc"] = np.zeros((4, 2, 128, 128), f32)
    for bi in range(4):
        for u in range(2):
            for dl in range(8):
                for n in range(DS):
                    sh["w_selbc"][bi, u, 32 * bi + 16 * u + n,
                                  16 * dl + n] = 1.0
    # out_proj with centering fold
    Pc = np.eye(DM, dtype=f32) - f32(1.0 / DM)
    WopT = (Pc @ out_proj_w).T                                 # [24,12]
    sh["w_op"] = np.zeros((128, 64), f32)
    for bi in range(4):
        sh["w_op"][32 * bi:32 * bi + 24, 16 * bi:16 * bi + 12] = WopT
    sh["w_ones12"] = np.zeros((128, 8), f32)
    sh["w_bc8"] = np.zeros((8, 128), f32)
    for b in range(8):
        sh["w_ones12"][16 * b:16 * b + 12, b] = f32(1.0 / DM)
        sh["w_bc8"][b, 16 * b:16 * b + 16] = 1.0
    # ffn
    W1p = (ffn_w1 * ln1_g[None, :]).T                          # [12,48]
    b1p = ffn_b1 + ffn_w1 @ ln1_b
    sh["w_ffn1"] = np.zeros((4, 128, 128), f32)
    sh["w_ffn2"] = np.zeros((4, 128, 32), f32)
    for q in range(4):
        for b2 in range(2):
            b = 2 * q + b2
            sh["w_ffn1"][q, 16 * b:16 * b + 12, 64 * b2:64 * b2 + 48] = W1p
            sh["w_ffn2"][q, 64 * b2:64 * b2 + 48,
                         16 * b2:16 * b2 + 12] = ffn_w2.T
    sh["w_pc"] = np.zeros((128, 128), f32)
    W1aT = (lin1_w[:, :DM] * ffn_ln_g[None, :]).T              # [12,12]
    W1bT = lin1_w[:, DM:].T
    sh["w_lin1a"] = np.zeros((128, 128), f32)
    sh["w_lin1b"] = np.zeros((128, 128), f32)
    for b in range(8):
        r = slice(16 * b, 16 * b + 12)
        sh["w_pc"][r, r] = Pc
        sh["w_lin1a"][r, r] = W1aT
        sh["w_lin1b"][r, r] = W1bT
    b1h = lin1_b + lin1_w[:, :DM] @ ffn_ln_b
    b2p = lin2_b - lin2_w.sum(axis=1)
    sh["w_lin2"] = np.zeros((2, 128, 128), f32)
    sh["w_lin3"] = np.zeros((2, 128, 4), f32)
    for g in range(2):
        for bi in range(4):
            b = 4 * g + bi
            sh["w_lin2"][g, 16 * b:16 * b + 12,
                         32 * bi:32 * bi + 20] = lin2_w.T
            sh["w_lin3"][g, 32 * bi:32 * bi + 20, bi] = lin3_w[0]
    sh["w_cnn"] = np.zeros((3, 96, 128), f16)
    for k in range(3):
        for b in range(8):
            sh["w_cnn"][k, 12 * b:12 * b + 12,
                        16 * b:16 * b + 12] = cnn_w[:, :, k].T.astype(f16)
    # scan masks and A scales
    sh["w_mask"] = np.zeros((3, 128, 32), np.float32)
    sh["sc_negA"] = np.zeros((3, 128, 1), f32)
    Asc = -np.exp(A_log)                                       # [24,16]
    for t in range(3):
        for dl in range(8):
            for n in range(DS):
                sh["w_mask"][t, 16 * dl + n, 8 * t + dl] = 1.0
                sh["sc_negA"][t, 16 * dl + n, 0] = Asc[8 * t + dl, n]
    sh["ident"] = np.eye(128, dtype=f16)
    sh["identj"] = np.eye(128, dtype=f16)[::-1].copy()

    def pack(v, blk, nblk):
        o = np.zeros(128, f32)
        for i in range(nblk):
            o[blk * i:blk * i + len(v)] = v
        return o

    vecs = np.zeros((128, 11), f32)
    bconv64 = np.zeros(64, f32)
    bconv64[0:24] = conv_b
    bconv64[32:56] = conv_b
    vecs[:, 0] = np.concatenate([bconv64, bconv64])
    vecs[:, 1] = pack(dt_b, 32, 4)
    vecs[:, 2] = pack(Dp, 32, 4)
    vecs[:, 3] = pack(ln1_g, 16, 8)
    vecs[:, 4] = pack(ln1_b, 16, 8)
    vecs[:, 5] = pack(b1p, 64, 2)
    vecs[:, 6] = pack(ffn_b2, 16, 8)
    vecs[:, 7] = pack(b1h, 16, 8)
    vecs[:, 8] = pack(b2p, 32, 4)
    vecs[:, 9] = pack(cnn_b, 16, 8)
    vecs[:, 10] = pack(b1p, 64, 2)
    sh["vecs"] = vecs
    sh["w_dp"] = np.zeros((4, 128, 32), f32)
    for bi in range(4):
        for c in range(DI):
            sh["w_dp"][bi, 32 * bi + c, c] = Dp[c]
    sh["b_out"] = np.full((8, 1), lin3_b[0], f32)
    sh["b_eps"] = np.full((8, 1), 1e-12, f32)
    # DFT matrices, tiled [mt, cs, kt, 128, 128]
    t_ = np.arange(L, dtype=np.float64)
    f_ = np.arange(NF, dtype=np.float64)
    ang = (2 * np.pi / L) * np.outer(f_, t_)
    wc = np.cos(ang)
    wsn = np.sin(ang)
    wc[1025:] = 0.0
    wsn[1025:] = 0.0
    wdft = np.zeros((NMT, 2, 128, NKT * 128), f16)
    for mt in range(NMT):
        for kt in range(NKT):
            blkc = wc[128 * mt:128 * mt + 128, 128 * kt:128 * kt + 128]
            blks = wsn[128 * mt:128 * mt + 128, 128 * kt:128 * kt + 128]
            wdft[mt, 0, :, 128 * kt:128 * kt + 128] = blkc.T.astype(f16)
            wdft[mt, 1, :, 128 * kt:128 * kt + 128] = blks.T.astype(f16)
    sh["wdft"] = wdft

    # pack weights per dtype
    import ml_dtypes as _md

    def _resolve(name):
        if name in sh:
            return sh[name]
        parts = name.split("_")
        idx = []
        while parts and parts[-1].isdigit():
            idx.insert(0, int(parts.pop()))
        return sh["_".join(parts)][tuple(idx)]

    def _pack(man, dt):
        ncols = sum(c for _, _, c in man)
        out = np.zeros((128, ncols), dt)
        o = 0
        for name, p, c in man:
            a = np.asarray(_resolve(name))
            assert a.shape == (p, c), (name, a.shape, p, c)
            out[0:p, o:o + c] = a.astype(dt)
            o += c
        return out

    packed = {"wpk_bf": _pack(MAN_BF, _md.bfloat16),
              "wpk_f16": _pack(MAN_F16, np.float16),
              "wpk_f32": _pack(MAN_F32, f32),
              "wdft": sh["wdft"]}
    sh = packed

    # per-core data
    per_core = []
    for c in range(NCORES):
        xl = x[BL * c:BL * c + BL]                             # [8,2048,12]
        xs = np.zeros((4, 96, N), f32)
        for j in range(4):
            for b2 in range(2):
                xb = xl[2 * j + b2]                            # [2048,12]
                for k in range(4):
                    shf = 3 - k
                    r0 = 48 * b2 + 12 * k
                    if shf == 0:
                        xs[j, r0:r0 + 12, :] = xb.T
                    else:
                        xs[j, r0:r0 + 12, shf:] = xb[:-shf].T
        xt = np.zeros((128, NKT * 96), f16)
        for kt in range(NKT):
            xt[:, 96 * kt:96 * kt + 96] = \
                xl[:, 128 * kt:128 * kt + 128].transpose(1, 0, 2) \
                .reshape(128, 96).astype(f16)
        import ml_dtypes as _md
        per_core.append({"xs": xs.astype(_md.bfloat16), "xt": xt})
    return sh, per_core


def kernel(**inputs):
    import ml_dtypes
    sh, per_core = _host_prep(inputs)
    if "nc" not in _CACHE:
        _CACHE["nc"] = _build_module()
    nc = _CACHE["nc"]
    in_maps = [{**sh, **pc} for pc in per_core]
    from concourse.bass_utils import run_bass_kernel_spmd
    res = run_bass_kernel_spmd(nc, in_maps, core_ids=list(range(NCORES)))
    outs = [res.results[c]["out"].reshape(BL) for c in range(NCORES)]
    return np.concatenate(outs).astype(np.float32)
